# revision 14
# baseline (speedup 1.0000x reference)
"""DiffTransformer layer on 8 TRN2 NeuronCores — collective-free shard.

Sharding: core c = (batch b=c//2, parity g=c%2). The 1024 query
positions of a batch are split into 16 blocks of 64; core g owns blocks
j with j%2==g (8 blocks, 512 queries). Causal work is balanced exactly:
both parities see key-tile counts {1..8} over their blocks, so a single
SPMD program serves all cores — only the diagonal-mask constant and the
gathered q-side inputs differ per core. Each core projects q for its
512 positions and k/v for the full sequence (the k/v duplication buys
the removal of both pair ReduceScatters of the previous design), runs
differential attention + per-head subln, out-projection straight into
SBUF, then the full FFN + residual + final RMSNorm for its positions.

Schedule: v(half0) first, then per-pt k/q projections interleaved
between the two attention passes of the previous pt so the ACT-engine
exp (the attention bottleneck) hides under projection matmuls; v(half1)
injected into the first iterations. Scores for key-tile t+1 are emitted
before pv of t to keep independent matmuls ahead of the exp-gated pv
pair in the strict FIFO PE queue. Softmax denominators ride the va
ones-column; division is deferred into the subln rsqrt; partition
broadcasts are selector matmuls with lambda folded host-side. DMAs are
batched into multi-tile transfers (each dma_start costs ~600ns serially
on the sync queue) and pools are scoped tightly so w1 can prefetch
during the attention tail; w2 streams under h2.
"""
import os
import sys
import numpy as np

for _p in ("/opt/trn_rl_repo", "/root/.axon_site/_ro/trn_rl_repo"):
    if os.path.isdir(_p) and _p not in sys.path:
        sys.path.append(_p)

B, S, D, H, HD, FF = 4, 1024, 1024, 16, 32, 4096
NCORES = 8
LAMBDA_INIT = 0.8 - 0.6 * float(np.exp(-0.3 * 12))
EPS = 1e-5
SCALE = float(HD) ** -0.5

SWAP16 = [((i + 16) % 32) for i in range(32)]

LAST_RESULT = None  # BassKernelResults of the most recent run (for test.py)
_PROGRAM = {}


def _build_program():
    import concourse.bacc as bacc
    import concourse.mybir as mybir
    from concourse import tile
    from contextlib import ExitStack

    dt = mybir.dt
    f32, f32r = dt.float32, dt.float32r
    bf16 = dt.bfloat16
    Alu = mybir.AluOpType
    Act = mybir.ActivationFunctionType

    nc = bacc.Bacc("TRN2", target_bir_lowering=False, debug=False,
                   num_devices=NCORES)

    P = 128
    xT = nc.declare_dram_parameter("xT", [D, S], bf16, isOutput=False)
    xQ = nc.declare_dram_parameter("xQ", [D, 512], bf16, isOutput=False)
    wqT = nc.declare_dram_parameter("wqT", [D, D], bf16, isOutput=False)
    wkT = nc.declare_dram_parameter("wkT", [D, D], bf16, isOutput=False)
    wvT = nc.declare_dram_parameter("wvT", [D, D], bf16, isOutput=False)
    woT = nc.declare_dram_parameter("woT", [D, D], bf16, isOutput=False)
    w1s = nc.declare_dram_parameter("w1s", [32, P, 1024], bf16, isOutput=False)
    w2T = nc.declare_dram_parameter("w2T", [FF, D], bf16, isOutput=False)
    b1c = nc.declare_dram_parameter("b1c", [P, 32], f32, isOutput=False)
    b2c = nc.declare_dram_parameter("b2c", [P, 8], f32, isOutput=False)
    rmswc = nc.declare_dram_parameter("rmswc", [P, 8], f32, isOutput=False)
    cosK = nc.declare_dram_parameter("cosK", [P, S], f32, isOutput=False)
    sinK = nc.declare_dram_parameter("sinK", [P, S], f32, isOutput=False)
    cosQ = nc.declare_dram_parameter("cosQ", [P, 512], f32, isOutput=False)
    sinQ = nc.declare_dram_parameter("sinQ", [P, 512], f32, isOutput=False)
    mdiag = nc.declare_dram_parameter("mdiag", [P, P], bf16, isOutput=False)
    hz1l = nc.declare_dram_parameter("hz1l", [P, P], f32r, isOutput=False)
    hz2 = nc.declare_dram_parameter("hz2", [P, P], f32r, isOutput=False)
    hselq = nc.declare_dram_parameter("hselq", [P, P], f32r, isOutput=False)
    hrstd = nc.declare_dram_parameter("hrstd", [16, 1024], f32r,
                                      isOutput=False)
    outT = nc.declare_dram_parameter("outT", [D, 512], f32, isOutput=True)

    def drearr(ap, k=8):
        return ap.rearrange("(k p) c -> p k c", k=k)

    with tile.TileContext(nc) as tc:
        with (
            tc.tile_pool(name="consts", bufs=1) as consts,
        ):
            _atstk = ExitStack()
            atp = _atstk.enter_context(tc.tile_pool(name="attn", bufs=1))
            _kqstk = ExitStack()
            kqp = _kqstk.enter_context(tc.tile_pool(name="kqva", bufs=1))
            _xvstk = ExitStack()
            xvp = _xvstk.enter_context(
                tc.tile_pool(name="xv", bufs=1, side="right"))
            _xqstk = ExitStack()
            xqp = _xqstk.enter_context(
                tc.tile_pool(name="xqw", bufs=1, side="right"))
            _wvstk = ExitStack()
            wvp = _wvstk.enter_context(
                tc.tile_pool(name="wv", bufs=1, side="right"))
            _wqkstk = ExitStack()
            wqkp = _wqkstk.enter_context(
                tc.tile_pool(name="wqk", bufs=1, side="right"))

            xt = xvp.tile([P, 8 * S], bf16, tag="xt")
            xt3 = xt[:].rearrange("p (k s) -> k p s", k=8)
            wv_sb = wvp.tile([P, 8 * 1024], bf16, tag="wv")
            wv4 = wv_sb[:].rearrange("p (u k c) -> u k p c", u=2, k=8)
            xq = xqp.tile([P, 8 * 512], bf16, tag="xq")
            xq3 = xq[:].rearrange("p (k s) -> k p s", k=8)
            wq_sb = wqkp.tile([P, 8 * 1024], bf16, tag="wq")
            wq3 = wq_sb[:].rearrange("p (k c) -> k p c", k=8)
            wk_sb = wqkp.tile([P, 8 * 1024], bf16, tag="wk")
            wk3 = wk_sb[:].rearrange("p (k c) -> k p c", k=8)

            # DMAs ordered by first use: v-proj leads, then k/q
            xtd = xt[:].rearrange("p (k s) -> p k s", k=8)
            wvd = wv_sb[:].rearrange("p (u k c) -> p u k c", u=2, k=8)
            wvs = wvT[:].rearrange("(k p) (u c) -> p u k c", k=8, u=2)
            nc.sync.dma_start(xtd[:, :, 0:512], drearr(xT[:])[:, :, 0:512])
            nc.sync.dma_start(wvd[:, 0], wvs[:, 0])
            nc.sync.dma_start(xtd[:, :, 512:1024],
                              drearr(xT[:])[:, :, 512:1024])
            nc.sync.dma_start(wvd[:, 1], wvs[:, 1])
            nc.sync.dma_start(wk_sb[:].rearrange("p (k c) -> p k c", k=8),
                              drearr(wkT[:]))
            nc.sync.dma_start(wq_sb[:].rearrange("p (k c) -> p k c", k=8),
                              drearr(wqT[:]))
            nc.sync.dma_start(xq[:].rearrange("p (k s) -> p k s", k=8),
                              drearr(xQ[:]))
            cosk_sb = xvp.tile([P, S], f32, tag="cosk")
            sink_sb = xvp.tile([P, S], f32, tag="sink")
            cosq_sb = xqp.tile([P, 512], f32, tag="cosq")
            sinq_sb = xqp.tile([P, 512], f32, tag="sinq")
            nc.sync.dma_start(cosk_sb[:], cosK[:])
            nc.sync.dma_start(sink_sb[:], sinK[:])
            nc.sync.dma_start(cosq_sb[:], cosQ[:])
            nc.sync.dma_start(sinq_sb[:], sinQ[:])

            md_sb = consts.tile([P, P], bf16, tag="md")
            hz1l_sb = consts.tile([P, P], f32r, tag="hz1l")
            hz2_sb = consts.tile([P, P], f32r, tag="hz2")
            hselq_sb = consts.tile([P, P], f32r, tag="hselq")
            hrstd_sb = consts.tile([16, 1024], f32r, tag="hrstd")
            b1_sb = consts.tile([P, 32], f32, tag="b1")
            b2_sb = consts.tile([P, 8], f32, tag="b2")
            rw_sb = consts.tile([P, 8], f32, tag="rw")
            nc.sync.dma_start(md_sb[:], mdiag[:])
            nc.sync.dma_start(hz1l_sb[:], hz1l[:])
            nc.sync.dma_start(hz2_sb[:], hz2[:])
            nc.sync.dma_start(hselq_sb[:], hselq[:])
            nc.sync.dma_start(hrstd_sb[:], hrstd[:])
            nc.sync.dma_start(b1_sb[:], b1c[:])
            nc.sync.dma_start(b2_sb[:], b2c[:])
            nc.sync.dma_start(rw_sb[:], rmswc[:])

            ones_f = consts.tile([P, 16], f32, tag="onesf")
            nc.vector.memset(ones_f[:], 1.0)
            ones_bf = consts.tile([P, 16], bf16, tag="onesb")
            nc.vector.tensor_copy(ones_bf[:], ones_f[:])
            eps_sb = consts.tile([1, 1], f32, tag="eps")
            nc.vector.memset(eps_sb[:], EPS)
            ones_r = consts.tile([P, 1], f32r, tag="onesr")
            nc.vector.tensor_copy(ones_r[:], ones_f[:, 0:1])
            onesw_f = consts.tile([1, P], f32, tag="oneswf")
            nc.vector.memset(onesw_f[:], 1.0)
            h1sel = consts.tile([1, P], f32r, tag="h1sel")
            nc.vector.tensor_copy(h1sel[:], onesw_f[:])

            qT = [kqp.tile([P, 512], bf16, tag=f"qT{i}", name=f"qT{i}")
                  for i in range(8)]
            kT = [kqp.tile([P, S], bf16, tag=f"kT{i}", name=f"kT{i}")
                  for i in range(8)]
            va = [kqp.tile([P, 16 * 65], bf16, tag=f"va{i}", name=f"va{i}")
                  for i in range(8)]
            aTr = [atp.tile([P, 512], bf16, tag=f"ar{i}", name=f"ar{i}")
                   for i in range(8)]

            _w1stk = ExitStack()
            _wostk = ExitStack()
            with (
                tc.tile_pool(name="st_ps", bufs=2, space="PSUM") as stp,
                tc.tile_pool(name="pv_ps", bufs=2, space="PSUM") as pvp,
                tc.tile_pool(name="ms_ps", bufs=1, space="PSUM") as msp,
                tc.tile_pool(name="bc_ps", bufs=1, space="PSUM") as bcp,
                tc.tile_pool(name="rtmp", bufs=2) as rtmp,
                tc.tile_pool(name="epool", bufs=3) as epool,
                tc.tile_pool(name="apool", bufs=1) as apool,
                tc.tile_pool(name="zpool", bufs=1) as zpool,
                tc.tile_pool(name="post", bufs=1) as post,
            ):
                aw = [(apool.tile([P, 512], bf16, tag=f"a1_{pt}",
                                  name=f"a1_{pt}"),
                       apool.tile([P, 512], bf16, tag=f"a2_{pt}",
                                  name=f"a2_{pt}")) for pt in range(8)]
                zpt = [zpool.tile([P, 512], f32r, tag=f"z{pt}",
                                  name=f"z{pt}") for pt in range(8)]
                poswq = {}
                for pt in range(8):
                    nc.vector.memset(zpt[pt][:].bitcast(f32), 0.0)
                ms = msp.tile([16, 512], f32, tag="ms", name="ms")

                def project_v(st, half):
                    ps = pvp.tile([P, 512], f32, tag="pv", name="vps")
                    for kd in range(8):
                        nc.tensor.matmul(
                            ps[:],
                            lhsT=xt3[kd, :, st * P:(st + 1) * P],
                            rhs=wv4[half, kd],
                            start=(kd == 0), stop=(kd == 7))
                    va3 = va[st][:].rearrange("p (h e) -> p h e", h=16, e=65)
                    nc.vector.tensor_copy(
                        va3[:, 8 * half:8 * half + 8, 0:64],
                        ps[:].rearrange("p (h e) -> p h e", h=8, e=64))
                    nc.vector.tensor_copy(
                        va3[:, 8 * half:8 * half + 8, 64:65],
                        ones_bf[:, 8 * half:8 * half + 8]
                        .rearrange("p (h o) -> p h o", o=1))

                def rope(dst, ps, cos_sb, sin_sb, c0, n):
                    tmp = rtmp.tile([P, 512], f32, tag="rt", name="rt")
                    nc.vector.stream_shuffle(tmp[:, 0:n], ps, SWAP16)
                    nc.vector.tensor_tensor(dst, ps, cos_sb[:, c0:c0 + n],
                                            Alu.mult)
                    tmp2 = rtmp.tile([P, 512], bf16, tag="rt2", name="rt2")
                    nc.vector.tensor_tensor(tmp2[:, 0:n], tmp[:, 0:n],
                                            sin_sb[:, c0:c0 + n], Alu.mult)
                    nc.gpsimd.tensor_tensor(dst, dst, tmp2[:, 0:n], Alu.add)

                def project_k(mt):
                    ps = stp.tile([P, 1024], f32, tag="st", name="kps")
                    ps3 = ps[:].rearrange("p (c n) -> p c n", c=2, n=512)
                    for kd in range(8):
                        lh = wk3[kd, :, mt * P:(mt + 1) * P]
                        nc.tensor.matmul(ps3[:, 0, :], lhsT=lh,
                                         rhs=xt3[kd, :, 0:512],
                                         start=(kd == 0), stop=(kd == 7))
                        nc.tensor.matmul(ps3[:, 1, :], lhsT=lh,
                                         rhs=xt3[kd, :, 512:1024],
                                         start=(kd == 0), stop=(kd == 7))
                    for ch in range(2):
                        rope(kT[mt][:, ch * 512:(ch + 1) * 512],
                             ps3[:, ch, :], cosk_sb, sink_sb, ch * 512, 512)

                def project_q(mt):
                    ps = stp.tile([P, 1024], f32, tag="st", name="qps")
                    for kd in range(8):
                        nc.tensor.matmul(
                            ps[:, 0:512],
                            lhsT=wq3[kd, :, mt * P:(mt + 1) * P],
                            rhs=xq3[kd, :, :],
                            start=(kd == 0), stop=(kd == 7))
                    rope(qT[mt][:], ps[:, 0:512], cosq_sb, sinq_sb, 0, 512)

                md3 = md_sb[:].rearrange("p (g n) -> p g n", g=2)

                def attn_pass(pt, h, defers=()):
                    # head hv=2pt+h, branch pair rows (2h, 2h+1)*32 of tile
                    pvA = pvp.tile([65, 512], f32, tag="pv", name="pvA")
                    pvB = pvp.tile([65, 512], f32, tag="pv", name="pvB")
                    hv = 2 * pt + h
                    es = {}

                    def emit_scores(t):
                        j0 = 64 * t
                        st = stp.tile([P, 1024], f32, tag="st", name="st")
                        st3 = st[:].rearrange("p (g n) -> p g n", g=2, n=512)
                        for gi, g in enumerate((2 * h, 2 * h + 1)):
                            nc.tensor.matmul(
                                st3[:, gi, j0:],
                                lhsT=kT[pt][g * 32:(g + 1) * 32,
                                            t * P:(t + 1) * P],
                                rhs=qT[pt][g * 32:(g + 1) * 32, j0:],
                                start=True, stop=True,
                                tile_position=(g * 32, 0))
                        e = epool.tile([P, 1024], bf16, tag="e", name="e")
                        e3 = e[:].rearrange("p (g n) -> p g n", g=2, n=512)
                        nc.scalar.activation(e3[:, :, j0:], st3[:, :, j0:],
                                             Act.Exp, scale=SCALE)
                        nc.vector.tensor_tensor(
                            e3[:, :, j0:j0 + 64], e3[:, :, j0:j0 + 64],
                            md3, Alu.mult)
                        es[t] = e3

                    def emit_pv(t):
                        j0 = 64 * t
                        e3 = es.pop(t)
                        nc.tensor.matmul(
                            pvA[:, j0:], lhsT=va[t][:, hv * 65:hv * 65 + 65],
                            rhs=e3[:, 0, j0:],
                            start=(t == 0), stop=(t == 7))
                        nc.tensor.matmul(
                            pvB[:, j0:], lhsT=va[t][:, hv * 65:hv * 65 + 65],
                            rhs=e3[:, 1, j0:],
                            start=(t == 0), stop=(t == 7))

                    emit_scores(0)
                    emit_scores(1)
                    emit_scores(2)
                    dq = list(defers)
                    for t in range(8):
                        if t + 3 < 8:
                            emit_scores(t + 3)
                        emit_pv(t)
                        if dq and t in (2, 4, 6):
                            dq.pop(0)()
                    a1, a2 = aw[pt]
                    z = zpt[pt]
                    nc.vector.tensor_copy(a1[64 * h:64 * h + 64, :],
                                          pvA[0:64, :])
                    nc.vector.tensor_copy(z[32 * h:32 * h + 1, :],
                                          pvA[64:65, :])
                    nc.vector.tensor_copy(a2[64 * h:64 * h + 64, :],
                                          pvB[0:64, :])
                    nc.scalar.copy(z[64 + 32 * h:65 + 32 * h, :],
                                   pvB[64:65, :])

                def posw_pieces(pt):
                    # posw = Z2*A1 - lam*Z1*A2 ; sq = posw^2/8 ; ms rows.
                    # Split in three so the DVE work interleaves between
                    # the pv tiles of the next pass instead of blocking
                    # its first mask multiply.
                    a1, a2 = aw[pt]
                    z = zpt[pt]
                    hold = {}

                    def p1():
                        bc2 = bcp.tile([P, 512], f32, tag="bc", name="bc2")
                        nc.tensor.matmul(bc2[:], lhsT=hz2_sb[:],
                                         rhs=z[:], start=True, stop=True)
                        t1 = post.tile([P, 512], f32r, tag="t1")
                        nc.vector.tensor_tensor(t1[:], a1[:], bc2[:],
                                                Alu.mult)
                        hold["t1"] = t1

                    def p2():
                        bc1 = bcp.tile([P, 512], f32, tag="bc", name="bc1")
                        nc.tensor.matmul(bc1[:], lhsT=hz1l_sb[:],
                                         rhs=z[:], start=True, stop=True)
                        t2 = post.tile([P, 512], f32r, tag="t2")
                        nc.vector.tensor_tensor(t2[:], a2[:], bc1[:],
                                                Alu.mult)
                        hold["t2"] = t2

                    def p3():
                        posw = post.tile([P, 512], bf16, tag="posw", bufs=8,
                                         name="posw")
                        nc.vector.tensor_tensor(posw[:], hold["t1"][:],
                                                hold["t2"][:], Alu.subtract)
                        sq = post.tile([P, 512], f32r, tag="t1", name="sq")
                        nc.vector.scalar_tensor_tensor(
                            sq[:], in0=posw[:], scalar=0.125, in1=posw[:],
                            op0=Alu.mult, op1=Alu.mult)
                        nc.tensor.matmul(
                            ms[:], lhsT=hselq_sb[:, 16 * pt:16 * pt + 16],
                            rhs=sq[:], start=(pt == 0), stop=(pt == 7))
                        poswq[pt] = posw

                    return (p1, p2, p3)

                def emit_rstd():
                    srt = rtmp.tile([16, 512], f32, tag="rt", name="srt")
                    nc.scalar.activation(srt[:], ms[:], Act.Sqrt,
                                         scale=1.0 / (1.0 - LAMBDA_INIT) ** 2)
                    rstd = rtmp.tile([16, 512], f32, tag="rt",
                                     name="rstd")
                    nc.vector.reciprocal_approx_fast(rstd[:], srt[:])
                    rstr = post.tile([16, 512], f32r, tag="t2", name="rstr")
                    nc.vector.tensor_copy(rstr[:], rstd[:])
                    return rstr[:]

                def emit_apply(pt, rstd):
                    pool = pvp if pt % 2 else bcp
                    tg = "pv" if pt % 2 else "bc"
                    bcr = pool.tile([P, 512], f32, tag=tg, name="bcr")
                    nc.tensor.matmul(bcr[:],
                                     lhsT=hrstd_sb[:, pt * P:(pt + 1) * P],
                                     rhs=rstd, start=True, stop=True)
                    nc.vector.tensor_tensor(attnT[pt][:], poswq[pt][:],
                                            bcr[:], Alu.mult)

                # ---- schedule -----------------------------------------
                for st in range(8):
                    project_v(st, 0)
                project_k(0)
                project_q(0)
                vh1 = {0: (0, 1, 2), 1: (3, 4, 5), 2: (6, 7)}
                kq = {0: (1, 2), 1: (3, 4, 5), 2: (6, 7)}
                for pt in range(8):
                    if pt == 3:
                        _wqkstk.close()
                        _wvstk.close()
                        _xqstk.close()
                        _xvstk.close()
                        wop = _wostk.enter_context(
                            tc.tile_pool(name="wo", bufs=1, side="right"))
                        wo_sb = wop.tile([P, 8 * 1024], bf16, tag="wo")
                        wo3 = wo_sb[:].rearrange("p (k c) -> k p c", k=8)
                        nc.sync.dma_start(
                            wo_sb[:].rearrange("p (k c) -> p k c", k=8),
                            drearr(woT[:]))
                        w1p = _w1stk.enter_context(
                            tc.tile_pool(name="w1p", bufs=1, side="right"))
                        w1_sb = w1p.tile([P, 32 * 1024], bf16, tag="w1")
                        w13 = w1_sb[:].rearrange("p (m c) -> m p c", m=32)
                        w1d = w1_sb[:].rearrange("p (m c) -> p m c", m=32)
                        for grp in range(4):
                            nc.sync.dma_start(
                                w1d[:, 8 * grp:8 * grp + 8, :],
                                w1s[8 * grp:8 * grp + 8, :, :]
                                .rearrange("m p c -> p m c"))
                    for st in vh1.get(pt, ()):
                        project_v(st, 1)
                    attn_pass(pt, 0,
                              defers=posw_pieces(pt - 1) if pt else ())
                    for mt in kq.get(pt, ()):
                        project_k(mt)
                        project_q(mt)
                    attn_pass(pt, 1)
                for cb in posw_pieces(7):
                    cb()
                rstd = emit_rstd()
                attnT = [kqp.tile([P, 512], bf16, tag=f"qT{i}",
                                  name=f"at{i}") for i in range(8)]
                # ---- apply + out-projection, kc-outer so wo matmuls
                # start as soon as the first attnT tile lands ----------
                wops = [stp.tile([P, 1024], f32, tag="st",
                                 name=f"wops{i}") for i in range(2)]
                for half in range(2):
                    for kc in range(8):
                        if half == 0:
                            emit_apply(kc, rstd)
                        for mo in range(4 * half, 4 * half + 4):
                            nc.tensor.matmul(
                                wops[(mo % 4) // 2][:, (mo % 2) * 512:
                                                    (mo % 2) * 512 + 512],
                                lhsT=wo3[kc, :, mo * P:(mo + 1) * P],
                                rhs=attnT[kc][:],
                                start=(kc == 0), stop=(kc == 7))
                    for mo in range(4 * half, 4 * half + 4):
                        nc.vector.tensor_copy(
                            aTr[mo][:],
                            wops[(mo % 4) // 2][:, (mo % 2) * 512:
                                                (mo % 2) * 512 + 512])

            _kqstk.close()

            # ---- FFN + residual + final RMS -------------------------
            with (
                tc.tile_pool(name="h1", bufs=1) as h1p,
                tc.tile_pool(name="w2p", bufs=4) as w2p,
                tc.tile_pool(name="yT", bufs=1) as ytp,
                tc.tile_pool(name="fin", bufs=2) as finp,
                tc.tile_pool(name="sm2", bufs=1) as sm2,
            ):
                h1 = [h1p.tile([P, 512], bf16, tag=f"h1_{i}", name=f"h1_{i}")
                      for i in range(32)]
                with tc.tile_pool(name="h1_ps", bufs=4, space="PSUM") as h1ps:
                    for mf in range(32):
                        ps = h1ps.tile([P, 512], f32, tag="h1ps",
                                       name="h1ps")
                        for kd in range(8):
                            nc.tensor.matmul(
                                ps[:], lhsT=w13[mf, :, kd * P:(kd + 1) * P],
                                rhs=aTr[kd][:], start=(kd == 0),
                                stop=(kd == 7))
                        nc.scalar.activation(h1[mf][:], ps[:], Act.Relu,
                                             bias=b1_sb[:, mf:mf + 1])
                _w1stk.close()
                _wostk.close()

                # h2 in two mo-groups (re-streaming w2) so the first
                # group's residual+square+ms runs under the second
                # group's matmuls; final stt split across DVE/GpSimd
                yt = [ytp.tile([P, 512], f32, tag=f"y{i}", name=f"y{i}")
                      for i in range(8)]
                with (
                    tc.tile_pool(name="h2_ps", bufs=1, space="PSUM") as h2ps,
                    tc.tile_pool(name="rms_ps", bufs=1,
                                 space="PSUM") as rmsps,
                ):
                    ms_ps = rmsps.tile([P, 512], f32, tag="rmsps",
                                       name="rmsps")
                    for grp in range(2):
                        mos = list(range(4 * grp, 4 * grp + 4))
                        ps4 = {mo: h2ps.tile([P, 512], f32,
                                             tag=f"h2_{mo % 4}",
                                             name=f"h2_{mo}") for mo in mos}
                        for kf in range(32):
                            wt2 = w2p.tile([P, 1024], bf16, tag="w2t",
                                           name="w2t")
                            nc.sync.dma_start(wt2[:],
                                              w2T[kf * P:(kf + 1) * P, :])
                            for mo in mos:
                                nc.tensor.matmul(
                                    ps4[mo][:],
                                    lhsT=wt2[:, mo * P:(mo + 1) * P],
                                    rhs=h1[kf][:], start=(kf == 0),
                                    stop=(kf == 31))
                        for i, mo in enumerate(mos):
                            nc.vector.scalar_tensor_tensor(
                                yt[mo][:], in0=ps4[mo][:],
                                scalar=b2_sb[:, mo:mo + 1], in1=aTr[mo][:],
                                op0=Alu.add, op1=Alu.add)
                            sq = finp.tile([P, 512], f32r, tag="fsq",
                                           name="fsq")
                            nc.scalar.activation(sq[:], yt[mo][:],
                                                 Act.Square)
                            nc.tensor.matmul(ms_ps[0:1, :], lhsT=ones_r[:],
                                             rhs=sq[:], start=(mo == 0),
                                             stop=(mo == 7))
                    srt = sm2.tile([1, 512], f32, tag="fsrt")
                    nc.scalar.activation(srt[:], ms_ps[0:1, :], Act.Sqrt,
                                         scale=1.0 / 1024.0, bias=eps_sb[:])
                    rstd2 = sm2.tile([1, 512], f32, tag="frstd")
                    nc.vector.reciprocal_approx_fast(rstd2[:], srt[:])
                    rstr2 = sm2.tile([1, 512], f32r, tag="frstr")
                    nc.vector.tensor_copy(rstr2[:], rstd2[:])
                    bcr = rmsps.tile([P, 512], f32, tag="fbc", name="fbc")
                    nc.tensor.matmul(bcr[:], lhsT=h1sel[:], rhs=rstr2[:],
                                     start=True, stop=True)
                    for mo in range(8):
                        ot = finp.tile([P, 512], f32, tag="fot", name="fot",
                                       bufs=4)
                        nc.vector.scalar_tensor_tensor(
                            ot[:], in0=yt[mo][:], scalar=rw_sb[:, mo:mo + 1],
                            in1=bcr[:], op0=Alu.mult, op1=Alu.mult)
                        nc.sync.dma_start(outT[mo * P:(mo + 1) * P, :], ot[:])

            _atstk.close()

    nc.compile()
    return nc


def _qcols(g):
    # core-local column c -> global seq position
    return np.concatenate(
        [np.arange(128 * i + 64 * g, 128 * i + 64 * g + 64)
         for i in range(8)])


def _host_prep(inputs):
    import ml_dtypes
    bfloat16 = ml_dtypes.bfloat16
    x = np.asarray(inputs["x"], dtype=np.float32)
    Wq = np.asarray(inputs["Wq"], dtype=np.float32)
    Wk = np.asarray(inputs["Wk"], dtype=np.float32)
    Wv = np.asarray(inputs["Wv"], dtype=np.float32)
    Wo = np.asarray(inputs["Wo"], dtype=np.float32)
    W1 = np.asarray(inputs["W1"], dtype=np.float32)
    b1 = np.asarray(inputs["b1"], dtype=np.float32)
    W2 = np.asarray(inputs["W2"], dtype=np.float32)
    b2 = np.asarray(inputs["b2"], dtype=np.float32)
    rmsw = np.asarray(inputs["rms_weight"], dtype=np.float32)
    lam = float(np.exp(np.dot(np.asarray(inputs["lambda_q1"], np.float64),
                              np.asarray(inputs["lambda_k1"], np.float64)))
                - np.exp(np.dot(np.asarray(inputs["lambda_q2"], np.float64),
                                np.asarray(inputs["lambda_k2"], np.float64)))
                + LAMBDA_INIT)

    half = HD // 2
    freqs = (1.0 / (10000.0 ** (np.arange(half, dtype=np.float32)
                                / np.float32(half)))).astype(np.float32)
    ang = (np.arange(S, dtype=np.float32)[:, None] * freqs[None, :])
    cos16 = np.cos(ang.astype(np.float32)).T.astype(np.float32)
    sin16 = np.sin(ang.astype(np.float32)).T.astype(np.float32)
    cosK_full = np.ascontiguousarray(
        np.tile(np.concatenate([cos16, cos16], 0), (4, 1)))
    sinK_full = np.ascontiguousarray(
        np.tile(np.concatenate([-sin16, sin16], 0), (4, 1)))
    perm32 = np.concatenate([np.arange(0, 32, 2), np.arange(1, 32, 2)])
    permed = np.concatenate([c0 * 32 + perm32 for c0 in range(32)])

    wqT_h = np.ascontiguousarray(Wq[permed, :].T.astype(bfloat16))
    wkT_h = np.ascontiguousarray(Wk[permed, :].T.astype(bfloat16))
    wvT_h = np.ascontiguousarray(Wv.T.astype(bfloat16))
    woT_h = np.ascontiguousarray(Wo.T.astype(bfloat16))
    w1s = np.ascontiguousarray(
        W1.T.reshape(8, 128, 32, 128).transpose(2, 1, 0, 3)
        .reshape(32, 128, 1024).astype(bfloat16))
    w2T_h = np.ascontiguousarray(W2.T.astype(bfloat16))
    b1c = np.ascontiguousarray(b1.reshape(32, 128).T)
    b2c = np.ascontiguousarray(b2.reshape(8, 128).T)
    rmswc = np.ascontiguousarray(rmsw.reshape(8, 128).T)

    # selector constants (role-independent)
    hz1l = np.zeros((128, 128), np.float32)
    hz2 = np.zeros((128, 128), np.float32)
    for h in range(2):
        pcols = slice(64 * h, 64 * h + 64)
        hz1l[32 * h, pcols] = lam
        hz2[64 + 32 * h, pcols] = 1.0
    hselq = np.zeros((128, 128), np.float32)
    hrstd = np.zeros((16, 1024), np.float32)
    for pt in range(8):
        for h in range(2):
            rows = slice(64 * h, 64 * h + 64)
            hselq[rows, 16 * pt + 2 * pt + h] = 0.125
            hrstd[2 * pt + h, 128 * pt + 64 * h:128 * pt + 64 * h + 64] = 1.0

    # per-parity diagonal masks (keys r=0..127 down, q offset o=0..63)
    r = np.arange(128)[:, None]
    o = np.arange(64)[None, :]
    md_g = []
    for g in range(2):
        if g == 0:
            m = (r < 64) & (r <= o)
        else:
            m = (r < 64) | ((r - 64) <= o)
        md_g.append(np.ascontiguousarray(
            np.tile(m.astype(np.float32), (1, 2)).astype(bfloat16)))

    in_maps = []
    for c in range(NCORES):
        b, g = c // 2, c % 2
        qc = _qcols(g)
        xTb = x[b].T.astype(bfloat16)
        in_maps.append({
            "xT": np.ascontiguousarray(xTb),
            "xQ": np.ascontiguousarray(xTb[:, qc]),
            "wqT": wqT_h, "wkT": wkT_h, "wvT": wvT_h, "woT": woT_h,
            "w1s": w1s, "w2T": w2T_h,
            "b1c": b1c, "b2c": b2c, "rmswc": rmswc,
            "cosK": cosK_full, "sinK": sinK_full,
            "cosQ": np.ascontiguousarray(cosK_full[:, qc]),
            "sinQ": np.ascontiguousarray(sinK_full[:, qc]),
            "mdiag": md_g[g],
            "hz1l": hz1l, "hz2": hz2, "hselq": hselq, "hrstd": hrstd,
        })
    return in_maps


def kernel(**inputs):
    global LAST_RESULT
    from concourse.bass_utils import run_bass_kernel_spmd

    if "nc" not in _PROGRAM:
        _PROGRAM["nc"] = _build_program()
    nc = _PROGRAM["nc"]

    in_maps = _host_prep(inputs)
    trace = bool(int(os.environ.get("KERNEL_TRACE", "0")))
    res = run_bass_kernel_spmd(nc, in_maps, list(range(NCORES)), trace=trace)
    LAST_RESULT = res

    out = np.empty((B, S, D), np.float32)
    for c in range(NCORES):
        b, g = c // 2, c % 2
        out[b, _qcols(g), :] = res.results[c]["outT"].T
    return out


# revision 17
# speedup vs baseline: 1.0018x; 1.0018x over previous
"""DiffTransformer layer on 8 TRN2 NeuronCores — collective-free shard.

Sharding: core c = (batch b=c//2, parity g=c%2). The 1024 query
positions of a batch are split into 16 blocks of 64; core g owns blocks
j with j%2==g (8 blocks, 512 queries). Causal work is balanced exactly:
both parities see key-tile counts {1..8} over their blocks, so a single
SPMD program serves all cores — only the diagonal-mask constant and the
gathered q-side inputs differ per core. Each core projects q for its
512 positions and k/v for the full sequence (the k/v duplication buys
the removal of both pair ReduceScatters of the previous design), runs
differential attention + per-head subln, out-projection straight into
SBUF, then the full FFN + residual + final RMSNorm for its positions.

Schedule: v(half0) first, then per-pt k/q projections interleaved
between the two attention passes of the previous pt so the ACT-engine
exp (the attention bottleneck) hides under projection matmuls; v(half1)
injected into the first iterations. Scores for key-tile t+1 are emitted
before pv of t to keep independent matmuls ahead of the exp-gated pv
pair in the strict FIFO PE queue. Softmax denominators ride the va
ones-column; division is deferred into the subln rsqrt; partition
broadcasts are selector matmuls with lambda folded host-side. DMAs are
batched into multi-tile transfers (each dma_start costs ~600ns serially
on the sync queue) and pools are scoped tightly so w1 can prefetch
during the attention tail; w2 streams under h2.
"""
import os
import sys
import numpy as np

for _p in ("/opt/trn_rl_repo", "/root/.axon_site/_ro/trn_rl_repo"):
    if os.path.isdir(_p) and _p not in sys.path:
        sys.path.append(_p)

B, S, D, H, HD, FF = 4, 1024, 1024, 16, 32, 4096
NCORES = 8
LAMBDA_INIT = 0.8 - 0.6 * float(np.exp(-0.3 * 12))
EPS = 1e-5
SCALE = float(HD) ** -0.5

SWAP16 = [((i + 16) % 32) for i in range(32)]

LAST_RESULT = None  # BassKernelResults of the most recent run (for test.py)
_PROGRAM = {}


def _build_program():
    import concourse.bacc as bacc
    import concourse.mybir as mybir
    from concourse import tile
    from contextlib import ExitStack

    dt = mybir.dt
    f32, f32r = dt.float32, dt.float32r
    bf16 = dt.bfloat16
    Alu = mybir.AluOpType
    Act = mybir.ActivationFunctionType

    nc = bacc.Bacc("TRN2", target_bir_lowering=False, debug=False,
                   num_devices=NCORES)

    P = 128
    xT = nc.declare_dram_parameter("xT", [D, S], bf16, isOutput=False)
    xQ = nc.declare_dram_parameter("xQ", [D, 512], bf16, isOutput=False)
    wqT = nc.declare_dram_parameter("wqT", [D, D], bf16, isOutput=False)
    wkT = nc.declare_dram_parameter("wkT", [D, D], bf16, isOutput=False)
    wvT = nc.declare_dram_parameter("wvT", [D, D], bf16, isOutput=False)
    woT = nc.declare_dram_parameter("woT", [D, D], bf16, isOutput=False)
    w1s = nc.declare_dram_parameter("w1s", [32, P, 1024], bf16, isOutput=False)
    w2T = nc.declare_dram_parameter("w2T", [FF, D], bf16, isOutput=False)
    b1c = nc.declare_dram_parameter("b1c", [P, 32], f32, isOutput=False)
    b2c = nc.declare_dram_parameter("b2c", [P, 8], f32, isOutput=False)
    rmswc = nc.declare_dram_parameter("rmswc", [P, 8], f32, isOutput=False)
    cosK = nc.declare_dram_parameter("cosK", [P, S], f32, isOutput=False)
    sinK = nc.declare_dram_parameter("sinK", [P, S], f32, isOutput=False)
    cosQ = nc.declare_dram_parameter("cosQ", [P, 512], f32, isOutput=False)
    sinQ = nc.declare_dram_parameter("sinQ", [P, 512], f32, isOutput=False)
    mdiag = nc.declare_dram_parameter("mdiag", [64, P], bf16, isOutput=False)
    ident64 = nc.declare_dram_parameter("ident64", [64, 64], bf16,
                                        isOutput=False)
    hz1l = nc.declare_dram_parameter("hz1l", [P, P], f32r, isOutput=False)
    hz2 = nc.declare_dram_parameter("hz2", [P, P], f32r, isOutput=False)
    hselq = nc.declare_dram_parameter("hselq", [P, P], f32r, isOutput=False)
    hrstd = nc.declare_dram_parameter("hrstd", [16, 1024], f32r,
                                      isOutput=False)
    outT = nc.declare_dram_parameter("outT", [D, 512], f32, isOutput=True)

    def drearr(ap, k=8):
        return ap.rearrange("(k p) c -> p k c", k=k)

    with tile.TileContext(nc) as tc:
        with (
            tc.tile_pool(name="consts", bufs=1) as consts,
        ):
            _atstk = ExitStack()
            atp = _atstk.enter_context(tc.tile_pool(name="attn", bufs=1))
            _kqstk = ExitStack()
            kqp = _kqstk.enter_context(tc.tile_pool(name="kqva", bufs=1))
            _xvstk = ExitStack()
            xvp = _xvstk.enter_context(
                tc.tile_pool(name="xv", bufs=1, side="right"))
            _xqstk = ExitStack()
            xqp = _xqstk.enter_context(
                tc.tile_pool(name="xqw", bufs=1, side="right"))
            _wvstk = ExitStack()
            wvp = _wvstk.enter_context(
                tc.tile_pool(name="wv", bufs=1, side="right"))
            _wqkstk = ExitStack()
            wqkp = _wqkstk.enter_context(
                tc.tile_pool(name="wqk", bufs=1, side="right"))

            xt = xvp.tile([P, 8 * S], bf16, tag="xt")
            xt3 = xt[:].rearrange("p (k s) -> k p s", k=8)
            wv_sb = wvp.tile([P, 8 * 1024], bf16, tag="wv")
            wv4 = wv_sb[:].rearrange("p (u k c) -> u k p c", u=2, k=8)
            xq = xqp.tile([P, 8 * 512], bf16, tag="xq")
            xq3 = xq[:].rearrange("p (k s) -> k p s", k=8)
            wq_sb = wqkp.tile([P, 8 * 1024], bf16, tag="wq")
            wq3 = wq_sb[:].rearrange("p (k c) -> k p c", k=8)
            wk_sb = wqkp.tile([P, 8 * 1024], bf16, tag="wk")
            wk3 = wk_sb[:].rearrange("p (k c) -> k p c", k=8)

            # DMAs ordered by first use: v-proj leads, then k/q
            xtd = xt[:].rearrange("p (k s) -> p k s", k=8)
            wvd = wv_sb[:].rearrange("p (u k c) -> p u k c", u=2, k=8)
            wvs = wvT[:].rearrange("(k p) (u c) -> p u k c", k=8, u=2)
            nc.sync.dma_start(xtd[:, :, 0:512], drearr(xT[:])[:, :, 0:512])
            nc.sync.dma_start(wvd[:, 0], wvs[:, 0])
            nc.sync.dma_start(xtd[:, :, 512:1024],
                              drearr(xT[:])[:, :, 512:1024])
            nc.sync.dma_start(wvd[:, 1], wvs[:, 1])
            nc.sync.dma_start(wk_sb[:].rearrange("p (k c) -> p k c", k=8),
                              drearr(wkT[:]))
            nc.sync.dma_start(wq_sb[:].rearrange("p (k c) -> p k c", k=8),
                              drearr(wqT[:]))
            nc.sync.dma_start(xq[:].rearrange("p (k s) -> p k s", k=8),
                              drearr(xQ[:]))
            cosk_sb = xvp.tile([P, S], f32, tag="cosk")
            sink_sb = xvp.tile([P, S], f32, tag="sink")
            cosq_sb = xqp.tile([P, 512], f32, tag="cosq")
            sinq_sb = xqp.tile([P, 512], f32, tag="sinq")
            nc.sync.dma_start(cosk_sb[:], cosK[:])
            nc.sync.dma_start(sink_sb[:], sinK[:])
            nc.sync.dma_start(cosq_sb[:], cosQ[:])
            nc.sync.dma_start(sinq_sb[:], sinQ[:])

            md_sb = consts.tile([64, P], bf16, tag="md")
            id_sb = consts.tile([64, 64], bf16, tag="id64")
            nc.sync.dma_start(id_sb[:], ident64[:])
            hz1l_sb = consts.tile([P, P], f32r, tag="hz1l")
            hz2_sb = consts.tile([P, P], f32r, tag="hz2")
            hselq_sb = consts.tile([P, P], f32r, tag="hselq")
            hrstd_sb = consts.tile([16, 1024], f32r, tag="hrstd")
            b1_sb = consts.tile([P, 32], f32, tag="b1")
            b2_sb = consts.tile([P, 8], f32, tag="b2")
            rw_sb = consts.tile([P, 8], f32, tag="rw")
            nc.sync.dma_start(md_sb[:], mdiag[:])
            nc.sync.dma_start(hz1l_sb[:], hz1l[:])
            nc.sync.dma_start(hz2_sb[:], hz2[:])
            nc.sync.dma_start(hselq_sb[:], hselq[:])
            nc.sync.dma_start(hrstd_sb[:], hrstd[:])
            nc.sync.dma_start(b1_sb[:], b1c[:])
            nc.sync.dma_start(b2_sb[:], b2c[:])
            nc.sync.dma_start(rw_sb[:], rmswc[:])

            ones_f = consts.tile([P, 16], f32, tag="onesf")
            nc.vector.memset(ones_f[:], 1.0)
            ones_bf = consts.tile([P, 16], bf16, tag="onesb")
            nc.vector.tensor_copy(ones_bf[:], ones_f[:])
            eps_sb = consts.tile([1, 1], f32, tag="eps")
            nc.vector.memset(eps_sb[:], EPS)
            ones_r = consts.tile([P, 1], f32r, tag="onesr")
            nc.vector.tensor_copy(ones_r[:], ones_f[:, 0:1])
            onesw_f = consts.tile([1, P], f32, tag="oneswf")
            nc.vector.memset(onesw_f[:], 1.0)
            h1sel = consts.tile([1, P], f32r, tag="h1sel")
            nc.vector.tensor_copy(h1sel[:], onesw_f[:])

            qT = [kqp.tile([P, 512], bf16, tag=f"qT{i}", name=f"qT{i}")
                  for i in range(8)]
            kT = [kqp.tile([P, S], bf16, tag=f"kT{i}", name=f"kT{i}")
                  for i in range(8)]
            va = [kqp.tile([P, 16 * 65], bf16, tag=f"va{i}", name=f"va{i}")
                  for i in range(8)]
            aTr = [atp.tile([P, 512], bf16, tag=f"ar{i}", name=f"ar{i}")
                   for i in range(8)]

            _w1stk = ExitStack()
            _wostk = ExitStack()
            with (
                tc.tile_pool(name="st_ps", bufs=2, space="PSUM") as stp,
                tc.tile_pool(name="pv_ps", bufs=2, space="PSUM") as pvp,
                tc.tile_pool(name="ms_ps", bufs=1, space="PSUM") as msp,
                tc.tile_pool(name="bc_ps", bufs=1, space="PSUM") as bcp,
                tc.tile_pool(name="rtmp", bufs=2) as rtmp,
                tc.tile_pool(name="epool", bufs=3) as epool,
                tc.tile_pool(name="apool", bufs=1) as apool,
                tc.tile_pool(name="zpool", bufs=1) as zpool,
                tc.tile_pool(name="post", bufs=1) as post,
            ):
                aw = [(apool.tile([P, 512], bf16, tag=f"a1_{pt}",
                                  name=f"a1_{pt}"),
                       apool.tile([P, 512], bf16, tag=f"a2_{pt}",
                                  name=f"a2_{pt}")) for pt in range(8)]
                zpt = [zpool.tile([P, 512], f32r, tag=f"z{pt}",
                                  name=f"z{pt}") for pt in range(8)]
                poswq = {}
                for pt in range(8):
                    nc.vector.memset(zpt[pt][:].bitcast(f32), 0.0)
                ms = msp.tile([16, 512], f32, tag="ms", name="ms")

                def project_v(st, half):
                    ps = pvp.tile([P, 512], f32, tag="pv", name="vps")
                    for kd in range(8):
                        nc.tensor.matmul(
                            ps[:],
                            lhsT=xt3[kd, :, st * P:(st + 1) * P],
                            rhs=wv4[half, kd],
                            start=(kd == 0), stop=(kd == 7))
                    va3 = va[st][:].rearrange("p (h e) -> p h e", h=16, e=65)
                    nc.vector.tensor_copy(
                        va3[:, 8 * half:8 * half + 8, 0:64],
                        ps[:].rearrange("p (h e) -> p h e", h=8, e=64))
                    nc.vector.tensor_copy(
                        va3[:, 8 * half:8 * half + 8, 64:65],
                        ones_bf[:, 8 * half:8 * half + 8]
                        .rearrange("p (h o) -> p h o", o=1))

                def rope(dst, ps, cos_sb, sin_sb, c0, n):
                    # DVE reads the PSUM (shuffle+cos) so the slot frees
                    # fast; the sin product and the add run on GpSimd
                    tmp = rtmp.tile([P, 512], f32, tag="rt", name="rt")
                    nc.vector.stream_shuffle(tmp[:, 0:n], ps, SWAP16)
                    nc.vector.tensor_tensor(dst, ps, cos_sb[:, c0:c0 + n],
                                            Alu.mult)
                    tmp2 = rtmp.tile([P, 512], bf16, tag="rt2", name="rt2")
                    nc.gpsimd.tensor_tensor(tmp2[:, 0:n], tmp[:, 0:n],
                                            sin_sb[:, c0:c0 + n], Alu.mult)
                    nc.gpsimd.tensor_tensor(dst, dst, tmp2[:, 0:n], Alu.add)

                def project_k(mt):
                    ps = stp.tile([P, 1024], f32, tag="st", name="kps")
                    ps3 = ps[:].rearrange("p (c n) -> p c n", c=2, n=512)
                    for kd in range(8):
                        lh = wk3[kd, :, mt * P:(mt + 1) * P]
                        nc.tensor.matmul(ps3[:, 0, :], lhsT=lh,
                                         rhs=xt3[kd, :, 0:512],
                                         start=(kd == 0), stop=(kd == 7))
                        nc.tensor.matmul(ps3[:, 1, :], lhsT=lh,
                                         rhs=xt3[kd, :, 512:1024],
                                         start=(kd == 0), stop=(kd == 7))
                    for ch in range(2):
                        rope(kT[mt][:, ch * 512:(ch + 1) * 512],
                             ps3[:, ch, :], cosk_sb, sink_sb, ch * 512, 512)

                def project_q(mt):
                    ps = stp.tile([P, 1024], f32, tag="st", name="qps")
                    for kd in range(8):
                        nc.tensor.matmul(
                            ps[:, 0:512],
                            lhsT=wq3[kd, :, mt * P:(mt + 1) * P],
                            rhs=xq3[kd, :, :],
                            start=(kd == 0), stop=(kd == 7))
                    rope(qT[mt][:], ps[:, 0:512], cosq_sb, sinq_sb, 0, 512)

                def attn_pass(pt, h, defers=()):
                    # head hv=2pt+h, branch pair rows (2h, 2h+1)*32 of tile
                    pvA = pvp.tile([65, 512], f32, tag="pv", name="pvA")
                    pvB = pvp.tile([65, 512], f32, tag="pv", name="pvB")
                    hv = 2 * pt + h
                    es = {}

                    def emit_scores(t):
                        j0 = 64 * t
                        st = stp.tile([P, 1024], f32, tag="st", name="st")
                        st3 = st[:].rearrange("p (g n) -> p g n", g=2, n=512)
                        for gi, g in enumerate((2 * h, 2 * h + 1)):
                            nc.tensor.matmul(
                                st3[:, gi, j0:],
                                lhsT=kT[pt][g * 32:(g + 1) * 32,
                                            t * P:(t + 1) * P],
                                rhs=qT[pt][g * 32:(g + 1) * 32, j0:],
                                start=True, stop=False,
                                tile_position=(g * 32, 0))
                        # causal mask: add a -1e4 band on the diagonal
                        # block via a tiny matmul; exp then yields zeros
                        for gi in range(2):
                            nc.tensor.matmul(
                                st3[:, gi, j0:j0 + 64],
                                lhsT=md_sb[:], rhs=id_sb[:],
                                start=False, stop=True)
                        e = epool.tile([P, 1024], bf16, tag="e", name="e")
                        e3 = e[:].rearrange("p (g n) -> p g n", g=2, n=512)
                        nc.scalar.activation(e3[:, :, j0:], st3[:, :, j0:],
                                             Act.Exp, scale=SCALE)
                        es[t] = e3

                    def emit_pv(t):
                        j0 = 64 * t
                        e3 = es.pop(t)
                        nc.tensor.matmul(
                            pvA[:, j0:], lhsT=va[t][:, hv * 65:hv * 65 + 65],
                            rhs=e3[:, 0, j0:],
                            start=(t == 0), stop=(t == 7))
                        nc.tensor.matmul(
                            pvB[:, j0:], lhsT=va[t][:, hv * 65:hv * 65 + 65],
                            rhs=e3[:, 1, j0:],
                            start=(t == 0), stop=(t == 7))

                    emit_scores(0)
                    emit_scores(1)
                    emit_scores(2)
                    dq = list(defers)
                    for t in range(8):
                        if t + 3 < 8:
                            emit_scores(t + 3)
                        emit_pv(t)
                        if dq and t in (2, 4, 6):
                            dq.pop(0)()
                    a1, a2 = aw[pt]
                    z = zpt[pt]
                    nc.vector.tensor_copy(a1[64 * h:64 * h + 64, :],
                                          pvA[0:64, :])
                    nc.vector.tensor_copy(z[32 * h:32 * h + 1, :],
                                          pvA[64:65, :])
                    nc.vector.tensor_copy(a2[64 * h:64 * h + 64, :],
                                          pvB[0:64, :])
                    nc.scalar.copy(z[64 + 32 * h:65 + 32 * h, :],
                                   pvB[64:65, :])

                def posw_pieces(pt):
                    # posw = Z2*A1 - lam*Z1*A2 ; sq = posw^2/8 ; ms rows.
                    # Split in three so the DVE work interleaves between
                    # the pv tiles of the next pass instead of blocking
                    # its first mask multiply.
                    a1, a2 = aw[pt]
                    z = zpt[pt]
                    hold = {}

                    def p1():
                        bc2 = bcp.tile([P, 512], f32, tag="bc", name="bc2")
                        nc.tensor.matmul(bc2[:], lhsT=hz2_sb[:],
                                         rhs=z[:], start=True, stop=True)
                        t1 = post.tile([P, 512], f32r, tag="t1")
                        nc.vector.tensor_tensor(t1[:], a1[:], bc2[:],
                                                Alu.mult)
                        hold["t1"] = t1

                    def p2():
                        bc1 = bcp.tile([P, 512], f32, tag="bc", name="bc1")
                        nc.tensor.matmul(bc1[:], lhsT=hz1l_sb[:],
                                         rhs=z[:], start=True, stop=True)
                        t2 = post.tile([P, 512], f32r, tag="t2")
                        nc.vector.tensor_tensor(t2[:], a2[:], bc1[:],
                                                Alu.mult)
                        hold["t2"] = t2

                    def p3():
                        posw = post.tile([P, 512], bf16, tag="posw", bufs=8,
                                         name="posw")
                        nc.vector.tensor_tensor(posw[:], hold["t1"][:],
                                                hold["t2"][:], Alu.subtract)
                        sq = post.tile([P, 512], f32r, tag="t1", name="sq")
                        nc.vector.scalar_tensor_tensor(
                            sq[:], in0=posw[:], scalar=0.125, in1=posw[:],
                            op0=Alu.mult, op1=Alu.mult)
                        nc.tensor.matmul(
                            ms[:], lhsT=hselq_sb[:, 16 * pt:16 * pt + 16],
                            rhs=sq[:], start=(pt == 0), stop=(pt == 7))
                        poswq[pt] = posw

                    return (p1, p2, p3)

                def emit_rstd():
                    srt = rtmp.tile([16, 512], f32, tag="rt", name="srt")
                    nc.scalar.activation(srt[:], ms[:], Act.Sqrt,
                                         scale=1.0 / (1.0 - LAMBDA_INIT) ** 2)
                    rstd = rtmp.tile([16, 512], f32, tag="rt",
                                     name="rstd")
                    nc.vector.reciprocal_approx_fast(rstd[:], srt[:])
                    rstr = post.tile([16, 512], f32r, tag="t2", name="rstr")
                    nc.vector.tensor_copy(rstr[:], rstd[:])
                    return rstr[:]

                def emit_apply(pt, rstd):
                    pool = pvp if pt % 2 else bcp
                    tg = "pv" if pt % 2 else "bc"
                    bcr = pool.tile([P, 512], f32, tag=tg, name="bcr")
                    nc.tensor.matmul(bcr[:],
                                     lhsT=hrstd_sb[:, pt * P:(pt + 1) * P],
                                     rhs=rstd, start=True, stop=True)
                    nc.vector.tensor_tensor(attnT[pt][:], poswq[pt][:],
                                            bcr[:], Alu.mult)

                # ---- schedule -----------------------------------------
                for st in range(4):
                    project_v(st, 0)
                project_k(0)
                project_q(0)
                for st in range(4, 8):
                    project_v(st, 0)
                project_k(1)
                project_q(1)
                vh1 = {0: (0, 1, 2), 1: (3, 4, 5), 2: (6, 7)}
                kq = {0: (2, 3), 1: (4, 5), 2: (6, 7)}
                for pt in range(8):
                    if pt == 3:
                        _wqkstk.close()
                        _wvstk.close()
                        _xqstk.close()
                        _xvstk.close()
                        wop = _wostk.enter_context(
                            tc.tile_pool(name="wo", bufs=1, side="right"))
                        wo_sb = wop.tile([P, 8 * 1024], bf16, tag="wo")
                        wo3 = wo_sb[:].rearrange("p (k c) -> k p c", k=8)
                        nc.sync.dma_start(
                            wo_sb[:].rearrange("p (k c) -> p k c", k=8),
                            drearr(woT[:]))
                        w1p = _w1stk.enter_context(
                            tc.tile_pool(name="w1p", bufs=1, side="right"))
                        w1_sb = w1p.tile([P, 32 * 1024], bf16, tag="w1")
                        w13 = w1_sb[:].rearrange("p (m c) -> m p c", m=32)
                        w1d = w1_sb[:].rearrange("p (m c) -> p m c", m=32)
                        for grp in range(4):
                            nc.sync.dma_start(
                                w1d[:, 8 * grp:8 * grp + 8, :],
                                w1s[8 * grp:8 * grp + 8, :, :]
                                .rearrange("m p c -> p m c"))
                    for st in vh1.get(pt, ()):
                        project_v(st, 1)
                    attn_pass(pt, 0,
                              defers=posw_pieces(pt - 1) if pt else ())
                    for mt in kq.get(pt, ()):
                        project_k(mt)
                        project_q(mt)
                    attn_pass(pt, 1)
                for cb in posw_pieces(7):
                    cb()
                rstd = emit_rstd()
                attnT = [kqp.tile([P, 512], bf16, tag=f"qT{i}",
                                  name=f"at{i}") for i in range(8)]
                # ---- apply + out-projection, kc-outer so wo matmuls
                # start as soon as the first attnT tile lands ----------
                wops = [stp.tile([P, 1024], f32, tag="st",
                                 name=f"wops{i}") for i in range(2)]
                for half in range(2):
                    for kc in range(8):
                        if half == 0:
                            emit_apply(kc, rstd)
                        for mo in range(4 * half, 4 * half + 4):
                            nc.tensor.matmul(
                                wops[(mo % 4) // 2][:, (mo % 2) * 512:
                                                    (mo % 2) * 512 + 512],
                                lhsT=wo3[kc, :, mo * P:(mo + 1) * P],
                                rhs=attnT[kc][:],
                                start=(kc == 0), stop=(kc == 7))
                    for mo in range(4 * half, 4 * half + 4):
                        nc.vector.tensor_copy(
                            aTr[mo][:],
                            wops[(mo % 4) // 2][:, (mo % 2) * 512:
                                                (mo % 2) * 512 + 512])

            _kqstk.close()

            # ---- FFN + residual + final RMS -------------------------
            with (
                tc.tile_pool(name="h1", bufs=1) as h1p,
                tc.tile_pool(name="w2p", bufs=4) as w2p,
                tc.tile_pool(name="yT", bufs=1) as ytp,
                tc.tile_pool(name="fin", bufs=2) as finp,
                tc.tile_pool(name="sm2", bufs=1) as sm2,
            ):
                h1 = [h1p.tile([P, 512], bf16, tag=f"h1_{i}", name=f"h1_{i}")
                      for i in range(32)]
                with tc.tile_pool(name="h1_ps", bufs=4, space="PSUM") as h1ps:
                    for mf in range(32):
                        ps = h1ps.tile([P, 512], f32, tag="h1ps",
                                       name="h1ps")
                        for kd in range(8):
                            nc.tensor.matmul(
                                ps[:], lhsT=w13[mf, :, kd * P:(kd + 1) * P],
                                rhs=aTr[kd][:], start=(kd == 0),
                                stop=(kd == 7))
                        nc.scalar.activation(h1[mf][:], ps[:], Act.Relu,
                                             bias=b1_sb[:, mf:mf + 1])
                _w1stk.close()
                _wostk.close()

                # h2 in two mo-groups (re-streaming w2) so the first
                # group's residual+square+ms runs under the second
                # group's matmuls; final stt split across DVE/GpSimd
                yt = [ytp.tile([P, 512], f32, tag=f"y{i}", name=f"y{i}")
                      for i in range(8)]
                with (
                    tc.tile_pool(name="h2_ps", bufs=1, space="PSUM") as h2ps,
                    tc.tile_pool(name="rms_ps", bufs=1,
                                 space="PSUM") as rmsps,
                ):
                    ms_ps = rmsps.tile([P, 512], f32, tag="rmsps",
                                       name="rmsps")

                    def h2_grp(grp):
                        mos = list(range(4 * grp, 4 * grp + 4))
                        ps4 = {mo: h2ps.tile([P, 512], f32,
                                             tag=f"h2_{mo % 4}",
                                             name=f"h2_{mo}") for mo in mos}
                        for kf in range(32):
                            wt2 = w2p.tile([P, 1024], bf16, tag="w2t",
                                           name="w2t")
                            nc.sync.dma_start(wt2[:],
                                              w2T[kf * P:(kf + 1) * P, :])
                            for mo in mos:
                                nc.tensor.matmul(
                                    ps4[mo][:],
                                    lhsT=wt2[:, mo * P:(mo + 1) * P],
                                    rhs=h1[kf][:], start=(kf == 0),
                                    stop=(kf == 31))
                        return ps4

                    def h2_post(ps4, mos):
                        for mo in mos:
                            nc.vector.scalar_tensor_tensor(
                                yt[mo][:], in0=ps4[mo][:],
                                scalar=b2_sb[:, mo:mo + 1], in1=aTr[mo][:],
                                op0=Alu.add, op1=Alu.add)
                            sq = finp.tile([P, 512], f32r, tag="fsq",
                                           name="fsq")
                            nc.scalar.activation(sq[:], yt[mo][:],
                                                 Act.Square)
                            nc.tensor.matmul(ms_ps[0:1, :], lhsT=ones_r[:],
                                             rhs=sq[:], start=(mo == 0),
                                             stop=(mo == 7))

                    psA = h2_grp(0)
                    psB = h2_grp(1)
                    h2_post(psA, [0, 1, 2, 3])
                    h2_post(psB, [4, 5, 6, 7])
                    srt = sm2.tile([1, 512], f32, tag="fsrt")
                    nc.scalar.activation(srt[:], ms_ps[0:1, :], Act.Sqrt,
                                         scale=1.0 / 1024.0, bias=eps_sb[:])
                    rstd2 = sm2.tile([1, 512], f32, tag="frstd")
                    nc.vector.reciprocal_approx_fast(rstd2[:], srt[:])
                    rstr2 = sm2.tile([1, 512], f32r, tag="frstr")
                    nc.vector.tensor_copy(rstr2[:], rstd2[:])
                    bcr = rmsps.tile([P, 512], f32, tag="fbc", name="fbc")
                    nc.tensor.matmul(bcr[:], lhsT=h1sel[:], rhs=rstr2[:],
                                     start=True, stop=True)
                    for mo in range(8):
                        ot = finp.tile([P, 512], f32, tag="fot", name="fot",
                                       bufs=4)
                        nc.vector.scalar_tensor_tensor(
                            ot[:], in0=yt[mo][:], scalar=rw_sb[:, mo:mo + 1],
                            in1=bcr[:], op0=Alu.mult, op1=Alu.mult)
                        nc.sync.dma_start(outT[mo * P:(mo + 1) * P, :], ot[:])

            _atstk.close()

    nc.compile()
    return nc


def _qcols(g):
    # core-local column c -> global seq position
    return np.concatenate(
        [np.arange(128 * i + 64 * g, 128 * i + 64 * g + 64)
         for i in range(8)])


def _host_prep(inputs):
    import ml_dtypes
    bfloat16 = ml_dtypes.bfloat16
    x = np.asarray(inputs["x"], dtype=np.float32)
    Wq = np.asarray(inputs["Wq"], dtype=np.float32)
    Wk = np.asarray(inputs["Wk"], dtype=np.float32)
    Wv = np.asarray(inputs["Wv"], dtype=np.float32)
    Wo = np.asarray(inputs["Wo"], dtype=np.float32)
    W1 = np.asarray(inputs["W1"], dtype=np.float32)
    b1 = np.asarray(inputs["b1"], dtype=np.float32)
    W2 = np.asarray(inputs["W2"], dtype=np.float32)
    b2 = np.asarray(inputs["b2"], dtype=np.float32)
    rmsw = np.asarray(inputs["rms_weight"], dtype=np.float32)
    lam = float(np.exp(np.dot(np.asarray(inputs["lambda_q1"], np.float64),
                              np.asarray(inputs["lambda_k1"], np.float64)))
                - np.exp(np.dot(np.asarray(inputs["lambda_q2"], np.float64),
                                np.asarray(inputs["lambda_k2"], np.float64)))
                + LAMBDA_INIT)

    half = HD // 2
    freqs = (1.0 / (10000.0 ** (np.arange(half, dtype=np.float32)
                                / np.float32(half)))).astype(np.float32)
    ang = (np.arange(S, dtype=np.float32)[:, None] * freqs[None, :])
    cos16 = np.cos(ang.astype(np.float32)).T.astype(np.float32)
    sin16 = np.sin(ang.astype(np.float32)).T.astype(np.float32)
    cosK_full = np.ascontiguousarray(
        np.tile(np.concatenate([cos16, cos16], 0), (4, 1)))
    sinK_full = np.ascontiguousarray(
        np.tile(np.concatenate([-sin16, sin16], 0), (4, 1)))
    perm32 = np.concatenate([np.arange(0, 32, 2), np.arange(1, 32, 2)])
    permed = np.concatenate([c0 * 32 + perm32 for c0 in range(32)])

    wqT_h = np.ascontiguousarray(Wq[permed, :].T.astype(bfloat16))
    wkT_h = np.ascontiguousarray(Wk[permed, :].T.astype(bfloat16))
    wvT_h = np.ascontiguousarray(Wv.T.astype(bfloat16))
    woT_h = np.ascontiguousarray(Wo.T.astype(bfloat16))
    w1s = np.ascontiguousarray(
        W1.T.reshape(8, 128, 32, 128).transpose(2, 1, 0, 3)
        .reshape(32, 128, 1024).astype(bfloat16))
    w2T_h = np.ascontiguousarray(W2.T.astype(bfloat16))
    b1c = np.ascontiguousarray(b1.reshape(32, 128).T)
    b2c = np.ascontiguousarray(b2.reshape(8, 128).T)
    rmswc = np.ascontiguousarray(rmsw.reshape(8, 128).T)

    # selector constants (role-independent)
    hz1l = np.zeros((128, 128), np.float32)
    hz2 = np.zeros((128, 128), np.float32)
    for h in range(2):
        pcols = slice(64 * h, 64 * h + 64)
        hz1l[32 * h, pcols] = lam
        hz2[64 + 32 * h, pcols] = 1.0
    hselq = np.zeros((128, 128), np.float32)
    hrstd = np.zeros((16, 1024), np.float32)
    for pt in range(8):
        for h in range(2):
            rows = slice(64 * h, 64 * h + 64)
            hselq[rows, 16 * pt + 2 * pt + h] = 0.125
            hrstd[2 * pt + h, 128 * pt + 64 * h:128 * pt + 64 * h + 64] = 1.0

    # per-parity additive causal band: mband[o, r] = -1e4 where the
    # (key r, q offset o) slot of a diagonal tile must be masked
    r = np.arange(128)[None, :]
    o = np.arange(64)[:, None]
    md_g = []
    for g in range(2):
        if g == 0:
            keep = (r < 64) & (r <= o)
        else:
            keep = (r < 64) | ((r - 64) <= o)
        md_g.append(np.ascontiguousarray(
            np.where(keep, 0.0, -1e4).astype(np.float32).astype(bfloat16)))
    ident64 = np.ascontiguousarray(np.eye(64, dtype=np.float32)
                                   .astype(bfloat16))

    in_maps = []
    for c in range(NCORES):
        b, g = c // 2, c % 2
        qc = _qcols(g)
        xTb = x[b].T.astype(bfloat16)
        in_maps.append({
            "xT": np.ascontiguousarray(xTb),
            "xQ": np.ascontiguousarray(xTb[:, qc]),
            "wqT": wqT_h, "wkT": wkT_h, "wvT": wvT_h, "woT": woT_h,
            "w1s": w1s, "w2T": w2T_h,
            "b1c": b1c, "b2c": b2c, "rmswc": rmswc,
            "cosK": cosK_full, "sinK": sinK_full,
            "cosQ": np.ascontiguousarray(cosK_full[:, qc]),
            "sinQ": np.ascontiguousarray(sinK_full[:, qc]),
            "mdiag": md_g[g], "ident64": ident64,
            "hz1l": hz1l, "hz2": hz2, "hselq": hselq, "hrstd": hrstd,
        })
    return in_maps


def kernel(**inputs):
    global LAST_RESULT
    from concourse.bass_utils import run_bass_kernel_spmd

    if "nc" not in _PROGRAM:
        _PROGRAM["nc"] = _build_program()
    nc = _PROGRAM["nc"]

    in_maps = _host_prep(inputs)
    trace = bool(int(os.environ.get("KERNEL_TRACE", "0")))
    res = run_bass_kernel_spmd(nc, in_maps, list(range(NCORES)), trace=trace)
    LAST_RESULT = res

    out = np.empty((B, S, D), np.float32)
    for c in range(NCORES):
        b, g = c // 2, c % 2
        out[b, _qcols(g), :] = res.results[c]["outT"].T
    return out


# revision 18
# speedup vs baseline: 1.0171x; 1.0153x over previous
"""DiffTransformer layer on 8 TRN2 NeuronCores — collective-free shard.

Sharding: core c = (batch b=c//2, parity g=c%2). The 1024 query
positions of a batch are split into 16 blocks of 64; core g owns blocks
j with j%2==g (8 blocks, 512 queries). Causal work is balanced exactly:
both parities see key-tile counts {1..8} over their blocks, so a single
SPMD program serves all cores — only the diagonal-mask constant and the
gathered q-side inputs differ per core. Each core projects q for its
512 positions and k/v for the full sequence (the k/v duplication buys
the removal of both pair ReduceScatters of the previous design), runs
differential attention + per-head subln, out-projection straight into
SBUF, then the full FFN + residual + final RMSNorm for its positions.

Schedule: v(half0) first, then per-pt k/q projections interleaved
between the two attention passes of the previous pt so the ACT-engine
exp (the attention bottleneck) hides under projection matmuls; v(half1)
injected into the first iterations. Scores for key-tile t+1 are emitted
before pv of t to keep independent matmuls ahead of the exp-gated pv
pair in the strict FIFO PE queue. Softmax denominators ride the va
ones-column; division is deferred into the subln rsqrt; partition
broadcasts are selector matmuls with lambda folded host-side. DMAs are
batched into multi-tile transfers (each dma_start costs ~600ns serially
on the sync queue) and pools are scoped tightly so w1 can prefetch
during the attention tail; w2 streams under h2.
"""
import os
import sys
import numpy as np

for _p in ("/opt/trn_rl_repo", "/root/.axon_site/_ro/trn_rl_repo"):
    if os.path.isdir(_p) and _p not in sys.path:
        sys.path.append(_p)

B, S, D, H, HD, FF = 4, 1024, 1024, 16, 32, 4096
NCORES = 8
LAMBDA_INIT = 0.8 - 0.6 * float(np.exp(-0.3 * 12))
EPS = 1e-5
SCALE = float(HD) ** -0.5

SWAP16 = [((i + 16) % 32) for i in range(32)]

LAST_RESULT = None  # BassKernelResults of the most recent run (for test.py)
_PROGRAM = {}


def _build_program():
    import concourse.bacc as bacc
    import concourse.mybir as mybir
    from concourse import tile
    from contextlib import ExitStack

    dt = mybir.dt
    f32, f32r = dt.float32, dt.float32r
    bf16 = dt.bfloat16
    Alu = mybir.AluOpType
    Act = mybir.ActivationFunctionType

    nc = bacc.Bacc("TRN2", target_bir_lowering=False, debug=False,
                   num_devices=NCORES)

    P = 128
    xT = nc.declare_dram_parameter("xT", [D, S], bf16, isOutput=False)
    xQ = nc.declare_dram_parameter("xQ", [D, 512], bf16, isOutput=False)
    wqT = nc.declare_dram_parameter("wqT", [D, D], bf16, isOutput=False)
    wkT = nc.declare_dram_parameter("wkT", [D, D], bf16, isOutput=False)
    wvT = nc.declare_dram_parameter("wvT", [D, D], bf16, isOutput=False)
    woT = nc.declare_dram_parameter("woT", [D, D], bf16, isOutput=False)
    w1s = nc.declare_dram_parameter("w1s", [32, P, 1024], bf16, isOutput=False)
    w2T = nc.declare_dram_parameter("w2T", [FF, D], bf16, isOutput=False)
    b1c = nc.declare_dram_parameter("b1c", [P, 32], f32, isOutput=False)
    b2c = nc.declare_dram_parameter("b2c", [P, 8], f32, isOutput=False)
    rmswc = nc.declare_dram_parameter("rmswc", [P, 8], f32, isOutput=False)
    cosK = nc.declare_dram_parameter("cosK", [P, S], f32, isOutput=False)
    sinK = nc.declare_dram_parameter("sinK", [P, S], f32, isOutput=False)
    cosQ = nc.declare_dram_parameter("cosQ", [P, 512], f32, isOutput=False)
    sinQ = nc.declare_dram_parameter("sinQ", [P, 512], f32, isOutput=False)
    mdiag = nc.declare_dram_parameter("mdiag", [64, P], bf16, isOutput=False)
    ident64 = nc.declare_dram_parameter("ident64", [64, 64], bf16,
                                        isOutput=False)
    hz1l = nc.declare_dram_parameter("hz1l", [P, P], f32r, isOutput=False)
    hz2 = nc.declare_dram_parameter("hz2", [P, P], f32r, isOutput=False)
    hselq = nc.declare_dram_parameter("hselq", [P, P], f32r, isOutput=False)
    hrstd = nc.declare_dram_parameter("hrstd", [16, 1024], f32r,
                                      isOutput=False)
    outT = nc.declare_dram_parameter("outT", [D, 512], f32, isOutput=True)

    def drearr(ap, k=8):
        return ap.rearrange("(k p) c -> p k c", k=k)

    with tile.TileContext(nc) as tc:
        with (
            tc.tile_pool(name="consts", bufs=1) as consts,
        ):
            _atstk = ExitStack()
            atp = _atstk.enter_context(tc.tile_pool(name="attn", bufs=1))
            _kqstk = ExitStack()
            kqp = _kqstk.enter_context(tc.tile_pool(name="kqva", bufs=1))
            _xvstk = ExitStack()
            xvp = _xvstk.enter_context(
                tc.tile_pool(name="xv", bufs=1, side="right"))
            _xqstk = ExitStack()
            xqp = _xqstk.enter_context(
                tc.tile_pool(name="xqw", bufs=1, side="right"))
            _wvstk = ExitStack()
            wvp = _wvstk.enter_context(
                tc.tile_pool(name="wv", bufs=1, side="right"))
            _wqkstk = ExitStack()
            wqkp = _wqkstk.enter_context(
                tc.tile_pool(name="wqk", bufs=1, side="right"))

            xt = xvp.tile([P, 8 * S], bf16, tag="xt")
            xt3 = xt[:].rearrange("p (k s) -> k p s", k=8)
            wv_sb = wvp.tile([P, 8 * 1024], bf16, tag="wv")
            wv4 = wv_sb[:].rearrange("p (u k c) -> u k p c", u=2, k=8)
            xq = xqp.tile([P, 8 * 512], bf16, tag="xq")
            xq3 = xq[:].rearrange("p (k s) -> k p s", k=8)
            wq_sb = wqkp.tile([P, 8 * 1024], bf16, tag="wq")
            wq3 = wq_sb[:].rearrange("p (k c) -> k p c", k=8)
            wk_sb = wqkp.tile([P, 8 * 1024], bf16, tag="wk")
            wk3 = wk_sb[:].rearrange("p (k c) -> k p c", k=8)

            # DMAs ordered by first use: v-proj leads, then k/q
            xtd = xt[:].rearrange("p (k s) -> p k s", k=8)
            wvd = wv_sb[:].rearrange("p (u k c) -> p u k c", u=2, k=8)
            wvs = wvT[:].rearrange("(k p) (u c) -> p u k c", k=8, u=2)
            nc.sync.dma_start(xtd[:, :, 0:512], drearr(xT[:])[:, :, 0:512])
            nc.sync.dma_start(wvd[:, 0], wvs[:, 0])
            nc.sync.dma_start(xtd[:, :, 512:1024],
                              drearr(xT[:])[:, :, 512:1024])
            nc.sync.dma_start(wvd[:, 1], wvs[:, 1])
            nc.sync.dma_start(wk_sb[:].rearrange("p (k c) -> p k c", k=8),
                              drearr(wkT[:]))
            nc.sync.dma_start(wq_sb[:].rearrange("p (k c) -> p k c", k=8),
                              drearr(wqT[:]))
            nc.sync.dma_start(xq[:].rearrange("p (k s) -> p k s", k=8),
                              drearr(xQ[:]))
            cosk_sb = xvp.tile([P, S], f32, tag="cosk")
            sink_sb = xvp.tile([P, S], f32, tag="sink")
            cosq_sb = xqp.tile([P, 512], f32, tag="cosq")
            sinq_sb = xqp.tile([P, 512], f32, tag="sinq")
            nc.sync.dma_start(cosk_sb[:], cosK[:])
            nc.sync.dma_start(sink_sb[:], sinK[:])
            nc.sync.dma_start(cosq_sb[:], cosQ[:])
            nc.sync.dma_start(sinq_sb[:], sinQ[:])

            md_sb = consts.tile([64, P], bf16, tag="md")
            id_sb = consts.tile([64, 64], bf16, tag="id64")
            nc.sync.dma_start(id_sb[:], ident64[:])
            hz1l_sb = consts.tile([P, P], f32r, tag="hz1l")
            hz2_sb = consts.tile([P, P], f32r, tag="hz2")
            hselq_sb = consts.tile([P, P], f32r, tag="hselq")
            hrstd_sb = consts.tile([16, 1024], f32r, tag="hrstd")
            b1_sb = consts.tile([P, 32], f32, tag="b1")
            b2_sb = consts.tile([P, 8], f32, tag="b2")
            rw_sb = consts.tile([P, 8], f32, tag="rw")
            nc.sync.dma_start(md_sb[:], mdiag[:])
            nc.sync.dma_start(hz1l_sb[:], hz1l[:])
            nc.sync.dma_start(hz2_sb[:], hz2[:])
            nc.sync.dma_start(hselq_sb[:], hselq[:])
            nc.sync.dma_start(hrstd_sb[:], hrstd[:])
            nc.sync.dma_start(b1_sb[:], b1c[:])
            nc.sync.dma_start(b2_sb[:], b2c[:])
            nc.sync.dma_start(rw_sb[:], rmswc[:])

            ones_f = consts.tile([P, 16], f32, tag="onesf")
            nc.vector.memset(ones_f[:], 1.0)
            ones_bf = consts.tile([P, 16], bf16, tag="onesb")
            nc.vector.tensor_copy(ones_bf[:], ones_f[:])
            eps_sb = consts.tile([1, 1], f32, tag="eps")
            nc.vector.memset(eps_sb[:], EPS)
            ones_r = consts.tile([P, 1], f32r, tag="onesr")
            nc.vector.tensor_copy(ones_r[:], ones_f[:, 0:1])
            onesw_f = consts.tile([1, P], f32, tag="oneswf")
            nc.vector.memset(onesw_f[:], 1.0)
            h1sel = consts.tile([1, P], f32r, tag="h1sel")
            nc.vector.tensor_copy(h1sel[:], onesw_f[:])

            qT = [kqp.tile([P, 512], bf16, tag=f"qT{i}", name=f"qT{i}")
                  for i in range(8)]
            kT = [kqp.tile([P, S], bf16, tag=f"kT{i}", name=f"kT{i}")
                  for i in range(8)]
            va = [kqp.tile([P, 16 * 65], bf16, tag=f"va{i}", name=f"va{i}")
                  for i in range(8)]
            aTr = [atp.tile([P, 512], bf16, tag=f"ar{i}", name=f"ar{i}")
                   for i in range(8)]

            _w1stk = ExitStack()
            _wostk = ExitStack()
            with (
                tc.tile_pool(name="st_ps", bufs=2, space="PSUM") as stp,
                tc.tile_pool(name="pv_ps", bufs=2, space="PSUM") as pvp,
                tc.tile_pool(name="ms_ps", bufs=1, space="PSUM") as msp,
                tc.tile_pool(name="bc_ps", bufs=1, space="PSUM") as bcp,
                tc.tile_pool(name="rtmp", bufs=2) as rtmp,
                tc.tile_pool(name="epool", bufs=3) as epool,
                tc.tile_pool(name="apool", bufs=1) as apool,
                tc.tile_pool(name="zpool", bufs=1) as zpool,
                tc.tile_pool(name="post", bufs=1) as post,
            ):
                aw = [(apool.tile([P, 512], bf16, tag=f"a1_{pt}",
                                  name=f"a1_{pt}"),
                       apool.tile([P, 512], bf16, tag=f"a2_{pt}",
                                  name=f"a2_{pt}")) for pt in range(8)]
                zpt = [zpool.tile([P, 512], f32r, tag=f"z{pt}",
                                  name=f"z{pt}") for pt in range(8)]
                poswq = {}
                for pt in range(8):
                    nc.vector.memset(zpt[pt][:].bitcast(f32), 0.0)
                ms = msp.tile([16, 512], f32, tag="ms", name="ms")

                def project_v(st, half):
                    ps = pvp.tile([P, 512], f32, tag="pv", name="vps")
                    for kd in range(8):
                        nc.tensor.matmul(
                            ps[:],
                            lhsT=xt3[kd, :, st * P:(st + 1) * P],
                            rhs=wv4[half, kd],
                            start=(kd == 0), stop=(kd == 7))
                    va3 = va[st][:].rearrange("p (h e) -> p h e", h=16, e=65)
                    nc.vector.tensor_copy(
                        va3[:, 8 * half:8 * half + 8, 0:64],
                        ps[:].rearrange("p (h e) -> p h e", h=8, e=64))
                    nc.vector.tensor_copy(
                        va3[:, 8 * half:8 * half + 8, 64:65],
                        ones_bf[:, 8 * half:8 * half + 8]
                        .rearrange("p (h o) -> p h o", o=1))

                def rope(dst, ps, cos_sb, sin_sb, c0, n):
                    # DVE reads the PSUM (shuffle+cos) so the slot frees
                    # fast; the sin product and the add run on GpSimd
                    tmp = rtmp.tile([P, 512], f32, tag="rt", name="rt")
                    nc.vector.stream_shuffle(tmp[:, 0:n], ps, SWAP16)
                    nc.vector.tensor_tensor(dst, ps, cos_sb[:, c0:c0 + n],
                                            Alu.mult)
                    tmp2 = rtmp.tile([P, 512], bf16, tag="rt2", name="rt2")
                    nc.gpsimd.tensor_tensor(tmp2[:, 0:n], tmp[:, 0:n],
                                            sin_sb[:, c0:c0 + n], Alu.mult)
                    nc.gpsimd.tensor_tensor(dst, dst, tmp2[:, 0:n], Alu.add)

                def project_k(mt):
                    ps = stp.tile([P, 1024], f32, tag="st", name="kps")
                    ps3 = ps[:].rearrange("p (c n) -> p c n", c=2, n=512)
                    for kd in range(8):
                        lh = wk3[kd, :, mt * P:(mt + 1) * P]
                        nc.tensor.matmul(ps3[:, 0, :], lhsT=lh,
                                         rhs=xt3[kd, :, 0:512],
                                         start=(kd == 0), stop=(kd == 7))
                        nc.tensor.matmul(ps3[:, 1, :], lhsT=lh,
                                         rhs=xt3[kd, :, 512:1024],
                                         start=(kd == 0), stop=(kd == 7))
                    for ch in range(2):
                        rope(kT[mt][:, ch * 512:(ch + 1) * 512],
                             ps3[:, ch, :], cosk_sb, sink_sb, ch * 512, 512)

                def project_q(mt):
                    ps = stp.tile([P, 1024], f32, tag="st", name="qps")
                    for kd in range(8):
                        nc.tensor.matmul(
                            ps[:, 0:512],
                            lhsT=wq3[kd, :, mt * P:(mt + 1) * P],
                            rhs=xq3[kd, :, :],
                            start=(kd == 0), stop=(kd == 7))
                    rope(qT[mt][:], ps[:, 0:512], cosq_sb, sinq_sb, 0, 512)

                def attn_pass(pt, h, defers=()):
                    # head hv=2pt+h, branch pair rows (2h, 2h+1)*32 of tile
                    pvA = pvp.tile([65, 512], f32, tag="pv", name="pvA")
                    pvB = pvp.tile([65, 512], f32, tag="pv", name="pvB")
                    hv = 2 * pt + h
                    es = {}

                    def emit_scores(t):
                        j0 = 64 * t
                        st = stp.tile([P, 1024], f32, tag="st", name="st")
                        st3 = st[:].rearrange("p (g n) -> p g n", g=2, n=512)
                        for gi, g in enumerate((2 * h, 2 * h + 1)):
                            nc.tensor.matmul(
                                st3[:, gi, j0:],
                                lhsT=kT[pt][g * 32:(g + 1) * 32,
                                            t * P:(t + 1) * P],
                                rhs=qT[pt][g * 32:(g + 1) * 32, j0:],
                                start=True, stop=False,
                                tile_position=(g * 32, 0))
                        # causal mask: add a -1e4 band on the diagonal
                        # block via a tiny matmul; exp then yields zeros
                        for gi in range(2):
                            nc.tensor.matmul(
                                st3[:, gi, j0:j0 + 64],
                                lhsT=md_sb[:], rhs=id_sb[:],
                                start=False, stop=True)
                        e = epool.tile([P, 1024], bf16, tag="e", name="e")
                        e3 = e[:].rearrange("p (g n) -> p g n", g=2, n=512)
                        nc.scalar.activation(e3[:, :, j0:], st3[:, :, j0:],
                                             Act.Exp, scale=SCALE)
                        es[t] = e3

                    def emit_pv(t):
                        j0 = 64 * t
                        e3 = es.pop(t)
                        nc.tensor.matmul(
                            pvA[:, j0:], lhsT=va[t][:, hv * 65:hv * 65 + 65],
                            rhs=e3[:, 0, j0:],
                            start=(t == 0), stop=(t == 7))
                        nc.tensor.matmul(
                            pvB[:, j0:], lhsT=va[t][:, hv * 65:hv * 65 + 65],
                            rhs=e3[:, 1, j0:],
                            start=(t == 0), stop=(t == 7))

                    emit_scores(0)
                    emit_scores(1)
                    emit_scores(2)
                    dq = list(defers)
                    for t in range(8):
                        if t + 3 < 8:
                            emit_scores(t + 3)
                        emit_pv(t)
                        if dq and t in (2, 4, 6):
                            dq.pop(0)()
                    a1, a2 = aw[pt]
                    z = zpt[pt]
                    nc.vector.tensor_copy(a1[64 * h:64 * h + 64, :],
                                          pvA[0:64, :])
                    nc.vector.tensor_copy(z[32 * h:32 * h + 1, :],
                                          pvA[64:65, :])
                    nc.vector.tensor_copy(a2[64 * h:64 * h + 64, :],
                                          pvB[0:64, :])
                    nc.scalar.copy(z[64 + 32 * h:65 + 32 * h, :],
                                   pvB[64:65, :])

                def posw_pieces(pt):
                    # posw = Z2*A1 - lam*Z1*A2 ; sq = posw^2/8 ; ms rows.
                    # Split in three so the DVE work interleaves between
                    # the pv tiles of the next pass instead of blocking
                    # its first mask multiply.
                    a1, a2 = aw[pt]
                    z = zpt[pt]
                    hold = {}

                    def p1():
                        bc2 = bcp.tile([P, 512], f32, tag="bc", name="bc2")
                        nc.tensor.matmul(bc2[:], lhsT=hz2_sb[:],
                                         rhs=z[:], start=True, stop=True)
                        t1 = post.tile([P, 512], f32r, tag="t1")
                        nc.vector.tensor_tensor(t1[:], a1[:], bc2[:],
                                                Alu.mult)
                        hold["t1"] = t1

                    def p2():
                        bc1 = bcp.tile([P, 512], f32, tag="bc", name="bc1")
                        nc.tensor.matmul(bc1[:], lhsT=hz1l_sb[:],
                                         rhs=z[:], start=True, stop=True)
                        t2 = post.tile([P, 512], f32r, tag="t2")
                        nc.vector.tensor_tensor(t2[:], a2[:], bc1[:],
                                                Alu.mult)
                        hold["t2"] = t2

                    def p3():
                        posw = post.tile([P, 512], bf16, tag="posw", bufs=8,
                                         name="posw")
                        nc.vector.tensor_tensor(posw[:], hold["t1"][:],
                                                hold["t2"][:], Alu.subtract)
                        sq = post.tile([P, 512], f32r, tag="t1", name="sq")
                        nc.vector.scalar_tensor_tensor(
                            sq[:], in0=posw[:], scalar=0.125, in1=posw[:],
                            op0=Alu.mult, op1=Alu.mult)
                        nc.tensor.matmul(
                            ms[:], lhsT=hselq_sb[:, 16 * pt:16 * pt + 16],
                            rhs=sq[:], start=(pt == 0), stop=(pt == 7))
                        poswq[pt] = posw

                    return (p1, p2, p3)

                def emit_rstd():
                    srt = rtmp.tile([16, 512], f32, tag="rt", name="srt")
                    nc.scalar.activation(srt[:], ms[:], Act.Sqrt,
                                         scale=1.0 / (1.0 - LAMBDA_INIT) ** 2)
                    rstd = rtmp.tile([16, 512], f32, tag="rt",
                                     name="rstd")
                    nc.vector.reciprocal_approx_fast(rstd[:], srt[:])
                    rstr = post.tile([16, 512], f32r, tag="t2", name="rstr")
                    nc.vector.tensor_copy(rstr[:], rstd[:])
                    return rstr[:]

                def emit_apply(pt, rstd):
                    pool = pvp if pt % 2 else bcp
                    tg = "pv" if pt % 2 else "bc"
                    bcr = pool.tile([P, 512], f32, tag=tg, name="bcr")
                    nc.tensor.matmul(bcr[:],
                                     lhsT=hrstd_sb[:, pt * P:(pt + 1) * P],
                                     rhs=rstd, start=True, stop=True)
                    nc.vector.tensor_tensor(attnT[pt][:], poswq[pt][:],
                                            bcr[:], Alu.mult)

                # ---- schedule -----------------------------------------
                for st in range(4):
                    project_v(st, 0)
                project_k(0)
                project_q(0)
                for st in range(4, 8):
                    project_v(st, 0)
                project_k(1)
                project_q(1)
                vh1 = {0: (0, 1, 2), 1: (3, 4, 5), 2: (6, 7)}
                kq = {0: (2,), 1: (3,), 2: (4,), 3: (5,), 4: (6,),
                      5: (7,)}
                for pt in range(8):
                    if pt == 6:
                        _wqkstk.close()
                        _wvstk.close()
                        _xqstk.close()
                        _xvstk.close()
                        wop = _wostk.enter_context(
                            tc.tile_pool(name="wo", bufs=1, side="right"))
                        wo_sb = wop.tile([P, 8 * 1024], bf16, tag="wo")
                        wo3 = wo_sb[:].rearrange("p (k c) -> k p c", k=8)
                        nc.sync.dma_start(
                            wo_sb[:].rearrange("p (k c) -> p k c", k=8),
                            drearr(woT[:]))
                        w1p = _w1stk.enter_context(
                            tc.tile_pool(name="w1p", bufs=1, side="right"))
                        w1_sb = w1p.tile([P, 32 * 1024], bf16, tag="w1")
                        w13 = w1_sb[:].rearrange("p (m c) -> m p c", m=32)
                        w1d = w1_sb[:].rearrange("p (m c) -> p m c", m=32)
                        for grp in range(4):
                            nc.sync.dma_start(
                                w1d[:, 8 * grp:8 * grp + 8, :],
                                w1s[8 * grp:8 * grp + 8, :, :]
                                .rearrange("m p c -> p m c"))
                    for st in vh1.get(pt, ()):
                        project_v(st, 1)
                    attn_pass(pt, 0,
                              defers=posw_pieces(pt - 1) if pt else ())
                    for mt in kq.get(pt, ()):
                        project_k(mt)
                        project_q(mt)
                    attn_pass(pt, 1)
                for cb in posw_pieces(7):
                    cb()
                rstd = emit_rstd()
                attnT = [kqp.tile([P, 512], bf16, tag=f"qT{i}",
                                  name=f"at{i}") for i in range(8)]
                # ---- apply + out-projection, kc-outer so wo matmuls
                # start as soon as the first attnT tile lands ----------
                wops = [stp.tile([P, 1024], f32, tag="st",
                                 name=f"wops{i}") for i in range(2)]
                for half in range(2):
                    for kc in range(8):
                        if half == 0:
                            emit_apply(kc, rstd)
                        for mo in range(4 * half, 4 * half + 4):
                            nc.tensor.matmul(
                                wops[(mo % 4) // 2][:, (mo % 2) * 512:
                                                    (mo % 2) * 512 + 512],
                                lhsT=wo3[kc, :, mo * P:(mo + 1) * P],
                                rhs=attnT[kc][:],
                                start=(kc == 0), stop=(kc == 7))
                    for mo in range(4 * half, 4 * half + 4):
                        nc.vector.tensor_copy(
                            aTr[mo][:],
                            wops[(mo % 4) // 2][:, (mo % 2) * 512:
                                                (mo % 2) * 512 + 512])

            _kqstk.close()

            # ---- FFN + residual + final RMS -------------------------
            with (
                tc.tile_pool(name="h1", bufs=1) as h1p,
                tc.tile_pool(name="w2p", bufs=4) as w2p,
                tc.tile_pool(name="yT", bufs=1) as ytp,
                tc.tile_pool(name="fin", bufs=2) as finp,
                tc.tile_pool(name="sm2", bufs=1) as sm2,
            ):
                h1 = [h1p.tile([P, 512], bf16, tag=f"h1_{i}", name=f"h1_{i}")
                      for i in range(32)]
                with tc.tile_pool(name="h1_ps", bufs=4, space="PSUM") as h1ps:
                    for mf in range(32):
                        ps = h1ps.tile([P, 512], f32, tag="h1ps",
                                       name="h1ps")
                        for kd in range(8):
                            nc.tensor.matmul(
                                ps[:], lhsT=w13[mf, :, kd * P:(kd + 1) * P],
                                rhs=aTr[kd][:], start=(kd == 0),
                                stop=(kd == 7))
                        nc.scalar.activation(h1[mf][:], ps[:], Act.Relu,
                                             bias=b1_sb[:, mf:mf + 1])
                _w1stk.close()
                _wostk.close()

                # h2 in two mo-groups (re-streaming w2) so the first
                # group's residual+square+ms runs under the second
                # group's matmuls; final stt split across DVE/GpSimd
                yt = [ytp.tile([P, 512], f32, tag=f"y{i}", name=f"y{i}")
                      for i in range(8)]
                with (
                    tc.tile_pool(name="h2_ps", bufs=1, space="PSUM") as h2ps,
                    tc.tile_pool(name="rms_ps", bufs=1,
                                 space="PSUM") as rmsps,
                ):
                    ms_ps = rmsps.tile([P, 512], f32, tag="rmsps",
                                       name="rmsps")

                    def h2_grp(grp):
                        mos = list(range(4 * grp, 4 * grp + 4))
                        ps4 = {mo: h2ps.tile([P, 512], f32,
                                             tag=f"h2_{mo % 4}",
                                             name=f"h2_{mo}") for mo in mos}
                        for kf in range(32):
                            wt2 = w2p.tile([P, 1024], bf16, tag="w2t",
                                           name="w2t")
                            nc.sync.dma_start(wt2[:],
                                              w2T[kf * P:(kf + 1) * P, :])
                            for mo in mos:
                                nc.tensor.matmul(
                                    ps4[mo][:],
                                    lhsT=wt2[:, mo * P:(mo + 1) * P],
                                    rhs=h1[kf][:], start=(kf == 0),
                                    stop=(kf == 31))
                        return ps4

                    def h2_post(ps4, mos):
                        for mo in mos:
                            nc.vector.scalar_tensor_tensor(
                                yt[mo][:], in0=ps4[mo][:],
                                scalar=b2_sb[:, mo:mo + 1], in1=aTr[mo][:],
                                op0=Alu.add, op1=Alu.add)
                            sq = finp.tile([P, 512], f32r, tag="fsq",
                                           name="fsq")
                            nc.scalar.activation(sq[:], yt[mo][:],
                                                 Act.Square)
                            nc.tensor.matmul(ms_ps[0:1, :], lhsT=ones_r[:],
                                             rhs=sq[:], start=(mo == 0),
                                             stop=(mo == 7))

                    psA = h2_grp(0)
                    psB = h2_grp(1)
                    h2_post(psA, [0, 1, 2, 3])
                    h2_post(psB, [4, 5, 6, 7])
                    srt = sm2.tile([1, 512], f32, tag="fsrt")
                    nc.scalar.activation(srt[:], ms_ps[0:1, :], Act.Sqrt,
                                         scale=1.0 / 1024.0, bias=eps_sb[:])
                    rstd2 = sm2.tile([1, 512], f32, tag="frstd")
                    nc.vector.reciprocal_approx_fast(rstd2[:], srt[:])
                    rstr2 = sm2.tile([1, 512], f32r, tag="frstr")
                    nc.vector.tensor_copy(rstr2[:], rstd2[:])
                    bcr = rmsps.tile([P, 512], f32, tag="fbc", name="fbc")
                    nc.tensor.matmul(bcr[:], lhsT=h1sel[:], rhs=rstr2[:],
                                     start=True, stop=True)
                    for mo in range(8):
                        ot = finp.tile([P, 512], f32, tag="fot", name="fot",
                                       bufs=4)
                        nc.vector.scalar_tensor_tensor(
                            ot[:], in0=yt[mo][:], scalar=rw_sb[:, mo:mo + 1],
                            in1=bcr[:], op0=Alu.mult, op1=Alu.mult)
                        nc.sync.dma_start(outT[mo * P:(mo + 1) * P, :], ot[:])

            _atstk.close()

    nc.compile()
    return nc


def _qcols(g):
    # core-local column c -> global seq position
    return np.concatenate(
        [np.arange(128 * i + 64 * g, 128 * i + 64 * g + 64)
         for i in range(8)])


def _host_prep(inputs):
    import ml_dtypes
    bfloat16 = ml_dtypes.bfloat16
    x = np.asarray(inputs["x"], dtype=np.float32)
    Wq = np.asarray(inputs["Wq"], dtype=np.float32)
    Wk = np.asarray(inputs["Wk"], dtype=np.float32)
    Wv = np.asarray(inputs["Wv"], dtype=np.float32)
    Wo = np.asarray(inputs["Wo"], dtype=np.float32)
    W1 = np.asarray(inputs["W1"], dtype=np.float32)
    b1 = np.asarray(inputs["b1"], dtype=np.float32)
    W2 = np.asarray(inputs["W2"], dtype=np.float32)
    b2 = np.asarray(inputs["b2"], dtype=np.float32)
    rmsw = np.asarray(inputs["rms_weight"], dtype=np.float32)
    lam = float(np.exp(np.dot(np.asarray(inputs["lambda_q1"], np.float64),
                              np.asarray(inputs["lambda_k1"], np.float64)))
                - np.exp(np.dot(np.asarray(inputs["lambda_q2"], np.float64),
                                np.asarray(inputs["lambda_k2"], np.float64)))
                + LAMBDA_INIT)

    half = HD // 2
    freqs = (1.0 / (10000.0 ** (np.arange(half, dtype=np.float32)
                                / np.float32(half)))).astype(np.float32)
    ang = (np.arange(S, dtype=np.float32)[:, None] * freqs[None, :])
    cos16 = np.cos(ang.astype(np.float32)).T.astype(np.float32)
    sin16 = np.sin(ang.astype(np.float32)).T.astype(np.float32)
    cosK_full = np.ascontiguousarray(
        np.tile(np.concatenate([cos16, cos16], 0), (4, 1)))
    sinK_full = np.ascontiguousarray(
        np.tile(np.concatenate([-sin16, sin16], 0), (4, 1)))
    perm32 = np.concatenate([np.arange(0, 32, 2), np.arange(1, 32, 2)])
    permed = np.concatenate([c0 * 32 + perm32 for c0 in range(32)])

    wqT_h = np.ascontiguousarray(Wq[permed, :].T.astype(bfloat16))
    wkT_h = np.ascontiguousarray(Wk[permed, :].T.astype(bfloat16))
    wvT_h = np.ascontiguousarray(Wv.T.astype(bfloat16))
    woT_h = np.ascontiguousarray(Wo.T.astype(bfloat16))
    w1s = np.ascontiguousarray(
        W1.T.reshape(8, 128, 32, 128).transpose(2, 1, 0, 3)
        .reshape(32, 128, 1024).astype(bfloat16))
    w2T_h = np.ascontiguousarray(W2.T.astype(bfloat16))
    b1c = np.ascontiguousarray(b1.reshape(32, 128).T)
    b2c = np.ascontiguousarray(b2.reshape(8, 128).T)
    rmswc = np.ascontiguousarray(rmsw.reshape(8, 128).T)

    # selector constants (role-independent)
    hz1l = np.zeros((128, 128), np.float32)
    hz2 = np.zeros((128, 128), np.float32)
    for h in range(2):
        pcols = slice(64 * h, 64 * h + 64)
        hz1l[32 * h, pcols] = lam
        hz2[64 + 32 * h, pcols] = 1.0
    hselq = np.zeros((128, 128), np.float32)
    hrstd = np.zeros((16, 1024), np.float32)
    for pt in range(8):
        for h in range(2):
            rows = slice(64 * h, 64 * h + 64)
            hselq[rows, 16 * pt + 2 * pt + h] = 0.125
            hrstd[2 * pt + h, 128 * pt + 64 * h:128 * pt + 64 * h + 64] = 1.0

    # per-parity additive causal band: mband[o, r] = -1e4 where the
    # (key r, q offset o) slot of a diagonal tile must be masked
    r = np.arange(128)[None, :]
    o = np.arange(64)[:, None]
    md_g = []
    for g in range(2):
        if g == 0:
            keep = (r < 64) & (r <= o)
        else:
            keep = (r < 64) | ((r - 64) <= o)
        md_g.append(np.ascontiguousarray(
            np.where(keep, 0.0, -1e4).astype(np.float32).astype(bfloat16)))
    ident64 = np.ascontiguousarray(np.eye(64, dtype=np.float32)
                                   .astype(bfloat16))

    in_maps = []
    for c in range(NCORES):
        b, g = c // 2, c % 2
        qc = _qcols(g)
        xTb = x[b].T.astype(bfloat16)
        in_maps.append({
            "xT": np.ascontiguousarray(xTb),
            "xQ": np.ascontiguousarray(xTb[:, qc]),
            "wqT": wqT_h, "wkT": wkT_h, "wvT": wvT_h, "woT": woT_h,
            "w1s": w1s, "w2T": w2T_h,
            "b1c": b1c, "b2c": b2c, "rmswc": rmswc,
            "cosK": cosK_full, "sinK": sinK_full,
            "cosQ": np.ascontiguousarray(cosK_full[:, qc]),
            "sinQ": np.ascontiguousarray(sinK_full[:, qc]),
            "mdiag": md_g[g], "ident64": ident64,
            "hz1l": hz1l, "hz2": hz2, "hselq": hselq, "hrstd": hrstd,
        })
    return in_maps


def kernel(**inputs):
    global LAST_RESULT
    from concourse.bass_utils import run_bass_kernel_spmd

    if "nc" not in _PROGRAM:
        _PROGRAM["nc"] = _build_program()
    nc = _PROGRAM["nc"]

    in_maps = _host_prep(inputs)
    trace = bool(int(os.environ.get("KERNEL_TRACE", "0")))
    res = run_bass_kernel_spmd(nc, in_maps, list(range(NCORES)), trace=trace)
    LAST_RESULT = res

    out = np.empty((B, S, D), np.float32)
    for c in range(NCORES):
        b, g = c // 2, c % 2
        out[b, _qcols(g), :] = res.results[c]["outT"].T
    return out


# revision 19
# speedup vs baseline: 1.0537x; 1.0360x over previous
"""DiffTransformer layer on 8 TRN2 NeuronCores.

Sharding: core c = (batch b=c//2, head-group g=c%2). Each core computes
q/k/v projections + differential attention for its 8 heads of its batch
(transposed [feature, seq] layout), a partial out-projection over its
512 attention channels, then pair ReduceScatters ([0,1],[2,3],...) sum
the two head-groups' partials and hand each core a 512-seq shard, on
which it runs the full FFN + residual + final RMSNorm.

vs baseline: bf16 data path everywhere (x/w/q/k/v/e/attn), causal mask
as a -300 additive band folded into the score PSUM via a PE matmul
(exp then yields ~0, no DVE mask), per-(pt,head-pass) attention with
batched 2-head exp from a 2-bank PSUM score tile, softmax denominators
via the va ones-column, division deferred into the subln RMS, all
partition broadcasts done as K=16/K=8 selector matmuls on the PE
(lambda folded into the selector host-side), rsqrt as exp(-.5*ln) to
keep ACT on one table set, wo emitted per D-half so the pair
ReduceScatter fires early, and the FFN h1 contraction split kd0-3 /
kd4-7 so the second RS overlaps the first half of h1.
"""
import os
import sys
import numpy as np

for _p in ("/opt/trn_rl_repo", "/root/.axon_site/_ro/trn_rl_repo"):
    if os.path.isdir(_p) and _p not in sys.path:
        sys.path.append(_p)

B, S, D, H, HD, FF = 4, 1024, 1024, 16, 32, 4096
NCORES = 8
LAMBDA_INIT = 0.8 - 0.6 * float(np.exp(-0.3 * 12))
EPS = 1e-5
SCALE = float(HD) ** -0.5
BAND = -300.0

SWAP16 = [((i + 16) % 32) for i in range(32)]

LAST_RESULT = None  # BassKernelResults of the most recent run (for test.py)
_PROGRAM = {}


def _kts(qc):
    # (k-tile index, diag-band offset or None=full) for a 512-wide q chunk
    if qc == 0:
        return [(0, 0), (1, 128), (2, 256), (3, 384)]
    return [(0, None), (1, None), (2, None), (3, None),
            (4, 0), (5, 128), (6, 256), (7, 384)]


def _build_program():
    import concourse.bacc as bacc
    import concourse.mybir as mybir
    from concourse import tile
    from contextlib import ExitStack

    dt = mybir.dt
    f32, f32r = dt.float32, dt.float32r
    bf16 = dt.bfloat16
    Alu = mybir.AluOpType
    Act = mybir.ActivationFunctionType

    nc = bacc.Bacc("TRN2", target_bir_lowering=False, debug=False,
                   num_devices=NCORES)

    P = 128
    xT = nc.declare_dram_parameter("xT", [D, S], bf16, isOutput=False)
    wqT = nc.declare_dram_parameter("wqT", [D, 512], bf16, isOutput=False)
    wkT = nc.declare_dram_parameter("wkT", [D, 512], bf16, isOutput=False)
    wvT = nc.declare_dram_parameter("wvT", [D, 512], bf16, isOutput=False)
    woT = nc.declare_dram_parameter("woT", [512, D], bf16, isOutput=False)
    w1s = nc.declare_dram_parameter("w1s", [32, P, 1024], bf16, isOutput=False)
    w2T = nc.declare_dram_parameter("w2T", [FF, D], bf16, isOutput=False)
    b1c = nc.declare_dram_parameter("b1c", [P, 32], f32, isOutput=False)
    b2c = nc.declare_dram_parameter("b2c", [P, 8], f32, isOutput=False)
    rmswc = nc.declare_dram_parameter("rmswc", [P, 8], f32, isOutput=False)
    cosT = nc.declare_dram_parameter("cosT", [P, S], f32, isOutput=False)
    sinS = nc.declare_dram_parameter("sinS", [P, S], f32, isOutput=False)
    mdiag = nc.declare_dram_parameter("mdiag", [P, 2 * P], bf16,
                                      isOutput=False)
    hz1l = nc.declare_dram_parameter("hz1l", [P, 128], f32r, isOutput=False)
    hz2 = nc.declare_dram_parameter("hz2", [P, 128], f32r, isOutput=False)
    hselq = nc.declare_dram_parameter("hselq", [P, 32], f32r, isOutput=False)
    hrstd = nc.declare_dram_parameter("hrstd", [8, 512], f32r, isOutput=False)
    outT = nc.declare_dram_parameter("outT", [D, 512], f32, isOutput=True)

    with tile.TileContext(nc) as tc:
        with (
            tc.tile_pool(name="consts", bufs=1) as consts,
            tc.tile_pool(name="dram", bufs=1, space="DRAM") as dram,
        ):
            # ---- constants -------------------------------------------
            md_sb = consts.tile([P, 2 * P], bf16, tag="md")
            hz1l_sb = consts.tile([P, 128], f32r, tag="hz1l")
            hz2_sb = consts.tile([P, 128], f32r, tag="hz2")
            hselq_sb = consts.tile([P, 32], f32r, tag="hselq")
            hrstd_sb = consts.tile([8, 512], f32r, tag="hrstd")
            b1_sb = consts.tile([P, 32], f32, tag="b1")
            b2_sb = consts.tile([P, 8], f32, tag="b2")
            rw_sb = consts.tile([P, 8], f32, tag="rw")
            wo_sb = [consts.tile([P, D], bf16, tag=f"wo{i}", name=f"wo{i}")
                     for i in range(4)]
            nc.sync.dma_start(md_sb[:], mdiag[:])
            nc.sync.dma_start(hz1l_sb[:], hz1l[:])
            nc.sync.dma_start(hz2_sb[:], hz2[:])
            nc.sync.dma_start(hselq_sb[:], hselq[:])
            nc.sync.dma_start(hrstd_sb[:], hrstd[:])
            nc.sync.dma_start(b1_sb[:], b1c[:])
            nc.sync.dma_start(b2_sb[:], b2c[:])
            nc.sync.dma_start(rw_sb[:], rmswc[:])
            for i in range(4):
                nc.sync.dma_start(wo_sb[i][:], woT[i * P:(i + 1) * P, :])
            ones_f = consts.tile([P, 8], f32, tag="onesf")
            nc.vector.memset(ones_f[:], 1.0)
            ones_bf = consts.tile([P, 8], bf16, tag="onesb")
            nc.vector.tensor_copy(ones_bf[:], ones_f[:])
            eps_sb = consts.tile([1, 1], f32, tag="eps")
            nc.vector.memset(eps_sb[:], EPS)
            ones_r = consts.tile([P, 1], f32r, tag="onesr")
            nc.vector.tensor_copy(ones_r[:], ones_f[:, 0:1])
            onesw_f = consts.tile([1, P], f32, tag="oneswf")
            nc.vector.memset(onesw_f[:], 1.0)
            h1sel = consts.tile([1, P], f32r, tag="h1sel")
            nc.vector.tensor_copy(h1sel[:], onesw_f[:])

            _stk = ExitStack()
            attnpool = _stk.enter_context(tc.tile_pool(name="attn", bufs=1))
            _qkstk = ExitStack()
            qkpool = _qkstk.enter_context(tc.tile_pool(name="qk", bufs=1))
            vapool = _qkstk.enter_context(tc.tile_pool(name="vaug", bufs=1))

            qT = [qkpool.tile([P, S], bf16, tag=f"qT{i}", name=f"qT{i}")
                  for i in range(4)]
            kT = [qkpool.tile([P, S], bf16, tag=f"kT{i}", name=f"kT{i}")
                  for i in range(4)]
            va = [vapool.tile([P, 8 * 65], bf16, tag=f"va{i}", name=f"va{i}")
                  for i in range(8)]
            attnT = [attnpool.tile([P, S], bf16, tag=f"at{i}", name=f"at{i}")
                     for i in range(4)]

            # [D-half][qc][512 D rows][512 seq]; each RS half contiguous
            po_dram = dram.tile([2, 2, 512, 512], bf16)
            rs_dram = dram.tile([2, 512, 512], bf16)

            # ---- phase 1+2: load xT / weights, project v then q,k ----
            _xwstk = ExitStack()
            xw = _xwstk.enter_context(tc.tile_pool(name="xw", bufs=1))
            with (
                tc.tile_pool(name="proj_ps", bufs=6, space="PSUM") as pps,
                tc.tile_pool(name="rtmp", bufs=4) as rtmp,
            ):
                xt = [xw.tile([P, S], bf16, tag=f"x{i}", name=f"x{i}")
                      for i in range(8)]
                cos_sb = xw.tile([P, S], f32, tag="cos")
                sin_sb = xw.tile([P, S], f32, tag="sin")
                nc.sync.dma_start(cos_sb[:], cosT[:])
                nc.sync.dma_start(sin_sb[:], sinS[:])
                wq_sb = [xw.tile([P, 512], bf16, tag=f"wq{i}", name=f"wqs{i}")
                         for i in range(8)]
                wk_sb = [xw.tile([P, 512], bf16, tag=f"wk{i}", name=f"wks{i}")
                         for i in range(8)]
                wv_sb = [xw.tile([P, 512], bf16, tag=f"wv{i}", name=f"wvs{i}")
                         for i in range(8)]
                for i in range(8):
                    nc.sync.dma_start(xt[i][:], xT[i * P:(i + 1) * P, :])
                    nc.sync.dma_start(wv_sb[i][:], wvT[i * P:(i + 1) * P, :])
                    nc.sync.dma_start(wq_sb[i][:], wqT[i * P:(i + 1) * P, :])
                    nc.sync.dma_start(wk_sb[i][:], wkT[i * P:(i + 1) * P, :])

                def project_v(st, pool=None, tag="ps"):
                    ps = (pool or pps).tile([P, 512], f32, tag=tag,
                                            name="ps")
                    for kd in range(8):
                        nc.tensor.matmul(
                            ps[:],
                            lhsT=xt[kd][:, st * P:(st + 1) * P],
                            rhs=wv_sb[kd][:],
                            start=(kd == 0), stop=(kd == 7))
                    va3 = va[st][:].rearrange("p (h e) -> p h e", h=8, e=65)
                    nc.vector.tensor_copy(
                        va3[:, :, 0:64],
                        ps[:].rearrange("p (h e) -> p h e", h=8, e=64))
                    nc.vector.tensor_copy(
                        va3[:, :, 64:65],
                        ones_bf[:].rearrange("p (h o) -> p h o", o=1))

                def project_qk(mt):
                    # both seq chunks share each lhsT slice
                    for wsb, dstT in ((wq_sb, qT), (wk_sb, kT)):
                        ps0 = pps.tile([P, 512], f32, tag="ps", name="ps")
                        ps1 = pps.tile([P, 512], f32, tag="ps", name="ps")
                        for kd in range(8):
                            lh = wsb[kd][:, mt * P:(mt + 1) * P]
                            nc.tensor.matmul(ps0[:], lhsT=lh,
                                             rhs=xt[kd][:, 0:512],
                                             start=(kd == 0), stop=(kd == 7))
                            nc.tensor.matmul(ps1[:], lhsT=lh,
                                             rhs=xt[kd][:, 512:1024],
                                             start=(kd == 0), stop=(kd == 7))
                        for nch, ps in ((0, ps0), (1, ps1)):
                            n0 = nch * 512
                            dst = dstT[mt][:, n0:n0 + 512]
                            tmp = rtmp.tile([P, 512], f32, tag="rt",
                                            name="rt")
                            nc.vector.stream_shuffle(tmp[:], ps[:], SWAP16)
                            nc.vector.tensor_tensor(
                                dst, ps[:], cos_sb[:, n0:n0 + 512], Alu.mult)
                            tmp2 = rtmp.tile([P, 512], bf16, tag="rt2",
                                             name="rt2")
                            nc.vector.tensor_tensor(
                                tmp2[:], tmp[:], sin_sb[:, n0:n0 + 512],
                                Alu.mult)
                            nc.gpsimd.tensor_tensor(dst, dst, tmp2[:],
                                                    Alu.add)

                # va[4..7] (k/v seq 512-1023, first used by qc1 kt>=4) are
                # deferred into the qc0 attention window as dense
                # full-array PE work that keeps the HAM un-throttled.
                for mt in range(4):
                    project_qk(mt)
                    project_v(mt)

            # ---- phase 3: differential attention ---------------------
            with (
                tc.tile_pool(name="st_ps", bufs=2, space="PSUM") as stp,
                tc.tile_pool(name="pv_ps", bufs=2, space="PSUM") as pvp,
                tc.tile_pool(name="ms_ps", bufs=1, space="PSUM") as msp,
                tc.tile_pool(name="bc_ps", bufs=1, space="PSUM") as bcp,
                tc.tile_pool(name="epool", bufs=3) as epool,
                tc.tile_pool(name="apool", bufs=3) as apool,
                tc.tile_pool(name="zpool", bufs=2) as zpool,
                tc.tile_pool(name="post", bufs=2) as post,
            ):
                zpt = {}
                ms = {}
                aw = {}

                def attn_pass(qc, pt, h):
                    # one head (2pt+h): branch pair gq=(2h, 2h+1).
                    # scores/exp for kt+1 are emitted BEFORE pv of kt so the
                    # strict-FIFO PE queue always has independent matmuls
                    # ahead of the exp-gated pv pair.
                    q0 = qc * 512
                    kts = _kts(qc)
                    last_kt = kts[-1][0]
                    pvA = pvp.tile([65, 512], f32, tag="pv", name="pvA")
                    pvB = pvp.tile([65, 512], f32, tag="pv", name="pvB")
                    hv = 2 * pt + h
                    es = {}

                    def emit_scores(kt, off):
                        j0 = 0 if off is None else off
                        st = stp.tile([P, 1024], f32, tag="st", name="st")
                        st3 = st[:].rearrange("p (g n) -> p g n", g=2, n=512)
                        for gi, g in enumerate((2 * h, 2 * h + 1)):
                            nc.tensor.matmul(
                                st3[:, gi, j0:],
                                lhsT=kT[pt][g * 32:(g + 1) * 32,
                                            kt * P:(kt + 1) * P],
                                rhs=qT[pt][g * 32:(g + 1) * 32,
                                           q0 + j0:q0 + 512],
                                start=True, stop=True,
                                tile_position=(g * 32, 0))
                        e = epool.tile([P, 1024], bf16, tag="e", name="e")
                        e3 = e[:].rearrange("p (g n) -> p g n", g=2, n=512)
                        nc.scalar.activation(e3[:, :, j0:], st3[:, :, j0:],
                                             Act.Exp, scale=SCALE)
                        if off is not None:
                            nc.vector.tensor_tensor(
                                e3[:, :, j0:j0 + P], e3[:, :, j0:j0 + P],
                                md_sb[:].rearrange("p (g n) -> p g n", g=2),
                                Alu.mult)
                        es[kt] = e3

                    def emit_pv(kt, off):
                        j0 = 0 if off is None else off
                        e3 = es.pop(kt)
                        nc.tensor.matmul(
                            pvA[:, j0:], lhsT=va[kt][:, hv * 65:hv * 65 + 65],
                            rhs=e3[:, 0, j0:],
                            start=(kt == 0), stop=(kt == last_kt))
                        nc.tensor.matmul(
                            pvB[:, j0:], lhsT=va[kt][:, hv * 65:hv * 65 + 65],
                            rhs=e3[:, 1, j0:],
                            start=(kt == 0), stop=(kt == last_kt))

                    emit_scores(*kts[0])
                    for i, (kt, off) in enumerate(kts):
                        if i + 1 < len(kts):
                            emit_scores(*kts[i + 1])
                        emit_pv(kt, off)
                    # evict: A rows (bf16) + Z rows at 32-aligned partitions.
                    # The last pass splits across DVE and the (by then idle)
                    # scalar engine to shorten the tail before wo/RS.
                    last = (qc == 1 and pt == 3)
                    a1, a2 = aw[(qc, pt)]
                    nc.vector.tensor_copy(a1[64 * h:64 * h + 64, :],
                                          pvA[0:64, :])
                    (nc.scalar.copy if last else nc.vector.tensor_copy)(
                        a2[64 * h:64 * h + 64, :], pvB[0:64, :])
                    z = zpt[(qc, pt)]
                    (nc.scalar.copy if last else nc.vector.tensor_copy)(
                        z[32 * h:32 * h + 1, :], pvA[64:65, :])
                    nc.vector.tensor_copy(z[64 + 32 * h:65 + 32 * h, :],
                                          pvB[64:65, :])

                def attn_pt(qc, pt):
                    if (qc, pt) not in aw:
                        aw[(qc, pt)] = (
                            apool.tile([P, 512], bf16, tag="a1", name="a1",
                                       bufs=5),
                            apool.tile([P, 512], bf16, tag="a2", name="a2",
                                       bufs=5))
                        z = zpool.tile([P, 512], f32r, tag="zpt", name="zpt",
                                       bufs=5)
                        nc.vector.memset(z[:].bitcast(f32), 0.0)
                        zpt[(qc, pt)] = z
                    attn_pass(qc, pt, 0)
                    attn_pass(qc, pt, 1)

                def emit_posw(qc, pt):
                    # posw = Z2*A1 - lam*Z1*A2 ; sq = posw^2/8 ; ms rows.
                    # The broadcasts read this pt's zpt directly (K=128
                    # selector) so posw pipelines with the next pt's pass.
                    a1, a2 = aw[(qc, pt)]
                    z = zpt[(qc, pt)]
                    bc2 = bcp.tile([P, 512], f32, tag="bc", name="bc2")
                    nc.tensor.matmul(bc2[:], lhsT=hz2_sb[:],
                                     rhs=z[:], start=True, stop=True)
                    t1 = post.tile([P, 512], f32r, tag="t1")
                    nc.vector.tensor_tensor(t1[:], a1[:], bc2[:], Alu.mult)
                    bc1 = bcp.tile([P, 512], f32, tag="bc", name="bc1")
                    nc.tensor.matmul(bc1[:], lhsT=hz1l_sb[:],
                                     rhs=z[:], start=True, stop=True)
                    t2 = post.tile([P, 512], f32r, tag="t2")
                    nc.vector.tensor_tensor(t2[:], a2[:], bc1[:], Alu.mult)
                    posw = post.tile([P, 512], f32r, tag="posw", bufs=5,
                                     name="posw")
                    nc.vector.tensor_tensor(posw[:], t1[:], t2[:],
                                            Alu.subtract)
                    sq = post.tile([P, 512], f32r, tag="sq", name="sq")
                    nc.vector.scalar_tensor_tensor(
                        sq[:], in0=posw[:], scalar=0.125, in1=posw[:],
                        op0=Alu.mult, op1=Alu.mult)
                    nc.tensor.matmul(ms[qc][:],
                                     lhsT=hselq_sb[:, 8 * pt:8 * pt + 8],
                                     rhs=sq[:], start=(pt == 0),
                                     stop=(pt == 3))
                    return posw

                def emit_rstd(qc):
                    # rstd rows = (1-li)*rsqrt(ms), [8,512]; the reference's
                    # +eps*(z1*z2)^2 term is ~4e-5 relative to ms and the
                    # (1-li) factor folds into the Rsqrt input scale.
                    srt = zpool.tile([8, 512], f32, tag="srt", name="srt")
                    nc.scalar.activation(srt[:], ms[qc][0:8, :], Act.Sqrt,
                                         scale=1.0 / (1.0 - LAMBDA_INIT) ** 2)
                    rstd = zpool.tile([8, 512], f32, tag="rstd", name="rstd")
                    nc.vector.reciprocal_approx_fast(rstd[:], srt[:])
                    rstr = zpool.tile([8, 512], f32r, tag="rstr",
                                      name="rstr")
                    nc.vector.tensor_copy(rstr[:], rstd[:])
                    return rstr[:]

                def emit_apply(qc, pt, posw, rstd):
                    q0 = qc * 512
                    bcr = bcp.tile([P, 512], f32, tag="bc", name="bcr")
                    nc.tensor.matmul(bcr[:],
                                     lhsT=hrstd_sb[:, pt * P:(pt + 1) * P],
                                     rhs=rstd, start=True, stop=True)
                    nc.vector.tensor_tensor(attnT[pt][:, q0:q0 + 512],
                                            posw[:], bcr[:], Alu.mult)

                poswq = {}
                for qc in range(2):
                    ms[qc] = msp.tile([8, 512], f32, tag="ms", name="ms")
                vdefer = {0: (4, 5), 1: (6,), 2: (7,), 3: ()}
                for pt in range(4):
                    attn_pt(0, pt)
                    for st in vdefer[pt]:
                        project_v(st, pool=bcp, tag="bc")
                    if pt > 0:
                        poswq[(0, pt - 1)] = emit_posw(0, pt - 1)
                # ---- phase 4 interleaved: wo by D-half + pair RS -----
                with tc.tile_pool(name="po_sb", bufs=4) as posb:
                    def emit_wo(dh, qc, pool=None):
                        q0 = qc * 512
                        for mo in range(4 * dh, 4 * dh + 4):
                            if pool is None:
                                pst = bcp.tile([P, 512], f32, tag="bc",
                                               name="wops")
                                ps = pst[:]
                            else:
                                pst = pool.tile([P, 1024], f32, tag="st",
                                                name="wops")
                                ps = pst[:, 0:512]
                            for kc in range(4):
                                nc.tensor.matmul(
                                    ps,
                                    lhsT=wo_sb[kc][:, mo * P:(mo + 1) * P],
                                    rhs=attnT[kc][:, q0:q0 + 512],
                                    start=(kc == 0), stop=(kc == 3))
                            po = posb.tile([P, 512], bf16, tag="po")
                            nc.vector.tensor_copy(po[:], ps)
                            nc.sync.dma_start(
                                po_dram[dh, qc,
                                        (mo % 4) * P:(mo % 4 + 1) * P, :],
                                po[:])

                    for pt in range(4):
                        attn_pt(1, pt)
                        if pt == 0:
                            poswq[(0, 3)] = emit_posw(0, 3)
                        else:
                            poswq[(1, pt - 1)] = emit_posw(1, pt - 1)
                        if pt == 1:
                            rstd0 = emit_rstd(0)
                            for p2 in range(4):
                                emit_apply(0, p2, poswq[(0, p2)], rstd0)
                        if pt == 2:
                            emit_wo(0, 0)
                        if pt == 3:
                            emit_wo(1, 0)
                    poswq[(1, 3)] = emit_posw(1, 3)
                    rstd1 = emit_rstd(1)
                    for p2 in range(4):
                        emit_apply(1, p2, poswq[(1, p2)], rstd1)
                    emit_wo(0, 1, pool=stp)
                    nc.gpsimd.collective_compute(
                        "ReduceScatter",
                        mybir.AluOpType.add,
                        replica_groups=[[0, 1], [2, 3], [4, 5], [6, 7]],
                        ins=[po_dram[0].opt()],
                        outs=[rs_dram[0].opt()],
                    )
                    emit_wo(1, 1, pool=stp)
                    nc.gpsimd.collective_compute(
                        "ReduceScatter",
                        mybir.AluOpType.add,
                        replica_groups=[[0, 1], [2, 3], [4, 5], [6, 7]],
                        ins=[po_dram[1].opt()],
                        outs=[rs_dram[1].opt()],
                    )

            _xwstk.close()
            _qkstk.close()
            _stk.close()

            # ---- phase 5: FFN + residual + final RMS on seq shard ----
            with (
                tc.tile_pool(name="aT", bufs=1) as atp,
                tc.tile_pool(name="h1", bufs=1) as h1p,
                tc.tile_pool(name="w1p", bufs=9) as w1p,
                tc.tile_pool(name="w2p", bufs=3) as w2p,
                tc.tile_pool(name="yT", bufs=1) as ytp,
                tc.tile_pool(name="fin", bufs=2) as finp,
                tc.tile_pool(name="sm2", bufs=1) as sm2,
            ):
                aTr = [atp.tile([P, 512], bf16, tag=f"ar{i}", name=f"ar{i}")
                       for i in range(8)]
                for i in range(8):
                    nc.sync.dma_start(
                        aTr[i][:],
                        rs_dram[i // 4, (i % 4) * P:(i % 4 + 1) * P, :])

                h1 = [h1p.tile([P, 512], bf16, tag=f"h1_{i}", name=f"h1_{i}")
                      for i in range(32)]
                with tc.tile_pool(name="h1_ps", bufs=8, space="PSUM") as h1ps:
                    wts = {}
                    pss = {}

                    def h1_first(mf):
                        wt = w1p.tile([P, 1024], bf16, tag="w1t", name="w1t")
                        nc.sync.dma_start(wt[:], w1s[mf, :, :])
                        ps = h1ps.tile([P, 512], f32, tag="h1ps",
                                       name="h1ps")
                        for kd in range(4):
                            nc.tensor.matmul(
                                ps[:], lhsT=wt[:, kd * P:(kd + 1) * P],
                                rhs=aTr[kd][:], start=(kd == 0), stop=False)
                        wts[mf], pss[mf] = wt, ps

                    def h1_second(mf):
                        wt, ps = wts.pop(mf), pss.pop(mf)
                        for kd in range(4, 8):
                            nc.tensor.matmul(
                                ps[:], lhsT=wt[:, kd * P:(kd + 1) * P],
                                rhs=aTr[kd][:], start=False, stop=(kd == 7))
                        nc.scalar.activation(h1[mf][:], ps[:], Act.Relu,
                                             bias=b1_sb[:, mf:mf + 1])

                    for mf in range(8):
                        h1_first(mf)
                    for mf in range(32):
                        h1_second(mf)
                        if mf + 8 < 32:
                            h1_first(mf + 8)

                # h2: 8 persistent PSUM accumulators, stream w2 tiles
                yt = [ytp.tile([P, 512], f32, tag=f"y{i}", name=f"y{i}")
                      for i in range(8)]
                with tc.tile_pool(name="h2_ps", bufs=1, space="PSUM") as h2ps:
                    ps8 = [h2ps.tile([P, 512], f32, tag=f"h2_{mo}",
                                     name=f"h2_{mo}") for mo in range(8)]
                    for kf in range(32):
                        wt2 = w2p.tile([P, 1024], bf16, tag="w2t",
                                       name="w2t")
                        nc.sync.dma_start(wt2[:], w2T[kf * P:(kf + 1) * P, :])
                        for mo in range(8):
                            nc.tensor.matmul(
                                ps8[mo][:], lhsT=wt2[:, mo * P:(mo + 1) * P],
                                rhs=h1[kf][:], start=(kf == 0),
                                stop=(kf == 31))
                    for mo in range(8):
                        nc.vector.scalar_tensor_tensor(
                            yt[mo][:], in0=ps8[mo][:],
                            scalar=b2_sb[:, mo:mo + 1], in1=aTr[mo][:],
                            op0=Alu.add, op1=Alu.add)

                with tc.tile_pool(name="rms_ps", bufs=1,
                                  space="PSUM") as rmsps:
                    ms_ps = rmsps.tile([P, 512], f32, tag="rmsps",
                                       name="rmsps")
                    for mo in range(8):
                        sq = finp.tile([P, 512], f32r, tag="fsq", name="fsq")
                        nc.scalar.activation(sq[:], yt[mo][:], Act.Square)
                        nc.tensor.matmul(ms_ps[0:1, :], lhsT=ones_r[:],
                                         rhs=sq[:], start=(mo == 0),
                                         stop=(mo == 7))
                    srt = sm2.tile([1, 512], f32, tag="fsrt")
                    nc.scalar.activation(srt[:], ms_ps[0:1, :], Act.Sqrt,
                                         scale=1.0 / 1024.0, bias=eps_sb[:])
                    rstd = sm2.tile([1, 512], f32, tag="frstd")
                    nc.vector.reciprocal_approx_fast(rstd[:], srt[:])
                    rstr = sm2.tile([1, 512], f32r, tag="frstr")
                    nc.vector.tensor_copy(rstr[:], rstd[:])
                    bcr = rmsps.tile([P, 512], f32, tag="fbc", name="fbc")
                    nc.tensor.matmul(bcr[:], lhsT=h1sel[:], rhs=rstr[:],
                                     start=True, stop=True)
                    for mo in range(8):
                        ot = finp.tile([P, 512], f32, tag="fot", name="fot")
                        nc.vector.scalar_tensor_tensor(
                            ot[:], in0=yt[mo][:], scalar=rw_sb[:, mo:mo + 1],
                            in1=bcr[:], op0=Alu.mult, op1=Alu.mult)
                        nc.sync.dma_start(outT[mo * P:(mo + 1) * P, :], ot[:])

    nc.compile()
    return nc


def _host_prep(inputs):
    import ml_dtypes
    bfloat16 = ml_dtypes.bfloat16
    x = np.asarray(inputs["x"], dtype=np.float32)
    Wq = np.asarray(inputs["Wq"], dtype=np.float32)
    Wk = np.asarray(inputs["Wk"], dtype=np.float32)
    Wv = np.asarray(inputs["Wv"], dtype=np.float32)
    Wo = np.asarray(inputs["Wo"], dtype=np.float32)
    W1 = np.asarray(inputs["W1"], dtype=np.float32)
    b1 = np.asarray(inputs["b1"], dtype=np.float32)
    W2 = np.asarray(inputs["W2"], dtype=np.float32)
    b2 = np.asarray(inputs["b2"], dtype=np.float32)
    rmsw = np.asarray(inputs["rms_weight"], dtype=np.float32)
    lam = float(np.exp(np.dot(np.asarray(inputs["lambda_q1"], np.float64),
                              np.asarray(inputs["lambda_k1"], np.float64)))
                - np.exp(np.dot(np.asarray(inputs["lambda_q2"], np.float64),
                                np.asarray(inputs["lambda_k2"], np.float64)))
                + LAMBDA_INIT)

    half = HD // 2
    freqs = (1.0 / (10000.0 ** (np.arange(half, dtype=np.float32)
                                / np.float32(half)))).astype(np.float32)
    ang = (np.arange(S, dtype=np.float32)[:, None] * freqs[None, :])
    cos16 = np.cos(ang.astype(np.float32)).T.astype(np.float32)
    sin16 = np.sin(ang.astype(np.float32)).T.astype(np.float32)

    cosT = np.ascontiguousarray(
        np.tile(np.concatenate([cos16, cos16], 0), (4, 1)))
    sinS = np.ascontiguousarray(
        np.tile(np.concatenate([-sin16, sin16], 0), (4, 1))).astype(np.float32)
    perm32 = np.concatenate([np.arange(0, 32, 2), np.arange(1, 32, 2)])

    # multiplicative causal mask for the diag band, tiled for both branches
    md = (np.arange(128)[:, None] <= np.arange(128)[None, :])
    mdiag = np.ascontiguousarray(
        np.tile(md.astype(np.float32), (1, 2)).astype(bfloat16))

    # selector matmuls: bc[p,s] = sum_k lhsT[k,p] * zq[k,s]
    # zq rows 0..7 = Z1 (key 2pt+h), 8..15 = Z2
    hz1l = np.zeros((128, 128), np.float32)
    hz2 = np.zeros((128, 128), np.float32)
    hselq = np.zeros((128, 32), np.float32)
    hrstd = np.zeros((8, 512), np.float32)
    for h in range(2):
        # bc[p, s] = zpt[32h(p), s]; zpt row 32h = Z1(h), 64+32h = Z2(h)
        pcols = slice(64 * h, 64 * h + 64)
        hz1l[32 * h, pcols] = lam
        hz2[64 + 32 * h, pcols] = 1.0
    for pt in range(4):
        for h in range(2):
            rows = slice(64 * h, 64 * h + 64)
            cols = slice(pt * 128 + 64 * h, pt * 128 + 64 * h + 64)
            hselq[rows, 8 * pt + 2 * pt + h] = 0.125
            hrstd[2 * pt + h, cols] = 1.0

    b1c = np.ascontiguousarray(b1.reshape(32, 128).T)
    b2c = np.ascontiguousarray(b2.reshape(8, 128).T)
    rmswc = np.ascontiguousarray(rmsw.reshape(8, 128).T)
    # w1s[mf][p, kd*128+j] = W1.T[kd*128+p, mf*128+j]
    w1s = np.ascontiguousarray(
        W1.T.reshape(8, 128, 32, 128).transpose(2, 1, 0, 3)
        .reshape(32, 128, 1024).astype(bfloat16))
    w2T = np.ascontiguousarray(W2.T.astype(bfloat16))

    in_maps = []
    for c in range(NCORES):
        b, g = c // 2, c % 2
        chans = np.arange(g * 512, (g + 1) * 512)
        permed = np.concatenate(
            [c0 * 32 + perm32 for c0 in range(g * 16, (g + 1) * 16)])
        in_maps.append({
            "xT": np.ascontiguousarray(x[b].T.astype(bfloat16)),
            "wqT": np.ascontiguousarray(Wq[permed, :].T.astype(bfloat16)),
            "wkT": np.ascontiguousarray(Wk[permed, :].T.astype(bfloat16)),
            "wvT": np.ascontiguousarray(Wv[chans, :].T.astype(bfloat16)),
            "woT": np.ascontiguousarray(Wo[:, chans].T.astype(bfloat16)),
            "w1s": w1s, "w2T": w2T,
            "b1c": b1c, "b2c": b2c, "rmswc": rmswc,
            "cosT": cosT, "sinS": sinS,
            "mdiag": mdiag,
            "hz1l": hz1l, "hz2": hz2, "hselq": hselq, "hrstd": hrstd,
        })
    return in_maps


def kernel(**inputs):
    global LAST_RESULT
    from concourse.bass_utils import run_bass_kernel_spmd

    if "nc" not in _PROGRAM:
        _PROGRAM["nc"] = _build_program()
    nc = _PROGRAM["nc"]

    in_maps = _host_prep(inputs)
    trace = bool(int(os.environ.get("KERNEL_TRACE", "0")))
    res = run_bass_kernel_spmd(nc, in_maps, list(range(NCORES)), trace=trace)
    LAST_RESULT = res

    out = np.empty((B, S, D), np.float32)
    for c in range(NCORES):
        b, g = c // 2, c % 2
        out[b, g * 512:(g + 1) * 512, :] = res.results[c]["outT"].T
    return out



# revision 20
# speedup vs baseline: 1.0982x; 1.0423x over previous
"""DiffTransformer layer on 8 TRN2 NeuronCores.

Sharding: core c = (batch b=c//2, head-group g=c%2). Each core computes
q/k/v projections + differential attention for its 8 heads of its batch
(transposed [feature, seq] layout), a partial out-projection over its
512 attention channels, then pair ReduceScatters ([0,1],[2,3],...) sum
the two head-groups' partials and hand each core a 512-seq shard, on
which it runs the full FFN + residual + final RMSNorm.

vs baseline: bf16 data path everywhere (x/w/q/k/v/e/attn), causal mask
as a -300 additive band folded into the score PSUM via a PE matmul
(exp then yields ~0, no DVE mask), per-(pt,head-pass) attention with
batched 2-head exp from a 2-bank PSUM score tile, softmax denominators
via the va ones-column, division deferred into the subln RMS, all
partition broadcasts done as K=16/K=8 selector matmuls on the PE
(lambda folded into the selector host-side), rsqrt as exp(-.5*ln) to
keep ACT on one table set, wo emitted per D-half so the pair
ReduceScatter fires early, and the FFN h1 contraction split kd0-3 /
kd4-7 so the second RS overlaps the first half of h1.
"""
import os
import sys
import numpy as np

for _p in ("/opt/trn_rl_repo", "/root/.axon_site/_ro/trn_rl_repo"):
    if os.path.isdir(_p) and _p not in sys.path:
        sys.path.append(_p)

B, S, D, H, HD, FF = 4, 1024, 1024, 16, 32, 4096
NCORES = 8
LAMBDA_INIT = 0.8 - 0.6 * float(np.exp(-0.3 * 12))
EPS = 1e-5
SCALE = float(HD) ** -0.5
BAND = -300.0

SWAP16 = [((i + 16) % 32) for i in range(32)]

LAST_RESULT = None  # BassKernelResults of the most recent run (for test.py)
_PROGRAM = {}


def _kts(qc):
    # (k-tile index, diag-band offset or None=full) for a 512-wide q chunk
    if qc == 0:
        return [(0, 0), (1, 128), (2, 256), (3, 384)]
    return [(0, None), (1, None), (2, None), (3, None),
            (4, 0), (5, 128), (6, 256), (7, 384)]


def _build_program():
    import concourse.bacc as bacc
    import concourse.mybir as mybir
    from concourse import tile
    from contextlib import ExitStack

    dt = mybir.dt
    f32, f32r = dt.float32, dt.float32r
    bf16 = dt.bfloat16
    Alu = mybir.AluOpType
    Act = mybir.ActivationFunctionType

    nc = bacc.Bacc("TRN2", target_bir_lowering=False, debug=False,
                   num_devices=NCORES)

    P = 128
    xT = nc.declare_dram_parameter("xT", [D, S], bf16, isOutput=False)
    wqT = nc.declare_dram_parameter("wqT", [D, 512], bf16, isOutput=False)
    wkT = nc.declare_dram_parameter("wkT", [D, 512], bf16, isOutput=False)
    wvT = nc.declare_dram_parameter("wvT", [D, 512], bf16, isOutput=False)
    woT = nc.declare_dram_parameter("woT", [512, D], bf16, isOutput=False)
    w1s = nc.declare_dram_parameter("w1s", [32, P, 1024], bf16, isOutput=False)
    w2T = nc.declare_dram_parameter("w2T", [FF, D], bf16, isOutput=False)
    b1c = nc.declare_dram_parameter("b1c", [P, 32], f32, isOutput=False)
    b2c = nc.declare_dram_parameter("b2c", [P, 8], f32, isOutput=False)
    rmswc = nc.declare_dram_parameter("rmswc", [P, 8], f32, isOutput=False)
    cosT = nc.declare_dram_parameter("cosT", [P, S], f32, isOutput=False)
    sinS = nc.declare_dram_parameter("sinS", [P, S], f32, isOutput=False)
    mdiag = nc.declare_dram_parameter("mdiag", [P, 2 * P], bf16,
                                      isOutput=False)
    hz1l = nc.declare_dram_parameter("hz1l", [P, 128], f32r, isOutput=False)
    hz2 = nc.declare_dram_parameter("hz2", [P, 128], f32r, isOutput=False)
    hselq = nc.declare_dram_parameter("hselq", [P, 32], f32r, isOutput=False)
    hrstd = nc.declare_dram_parameter("hrstd", [8, 512], f32r, isOutput=False)
    outT = nc.declare_dram_parameter("outT", [D, 512], f32, isOutput=True)

    with tile.TileContext(nc) as tc:
        with (
            tc.tile_pool(name="consts", bufs=1) as consts,
            tc.tile_pool(name="dram", bufs=1, space="DRAM") as dram,
        ):
            # ---- constants -------------------------------------------
            md_sb = consts.tile([P, 2 * P], bf16, tag="md")
            hz1l_sb = consts.tile([P, 128], f32r, tag="hz1l")
            hz2_sb = consts.tile([P, 128], f32r, tag="hz2")
            hselq_sb = consts.tile([P, 32], f32r, tag="hselq")
            hrstd_sb = consts.tile([8, 512], f32r, tag="hrstd")
            b1_sb = consts.tile([P, 32], f32, tag="b1")
            b2_sb = consts.tile([P, 8], f32, tag="b2")
            rw_sb = consts.tile([P, 8], f32, tag="rw")
            wo_sb = [consts.tile([P, D], bf16, tag=f"wo{i}", name=f"wo{i}")
                     for i in range(4)]
            ones_f = consts.tile([P, 8], f32, tag="onesf")
            nc.vector.memset(ones_f[:], 1.0)
            ones_bf = consts.tile([P, 8], bf16, tag="onesb")
            nc.vector.tensor_copy(ones_bf[:], ones_f[:])
            eps_sb = consts.tile([1, 1], f32, tag="eps")
            nc.vector.memset(eps_sb[:], EPS)
            ones_r = consts.tile([P, 1], f32r, tag="onesr")
            nc.vector.tensor_copy(ones_r[:], ones_f[:, 0:1])
            onesw_f = consts.tile([1, P], f32, tag="oneswf")
            nc.vector.memset(onesw_f[:], 1.0)
            h1sel = consts.tile([1, P], f32r, tag="h1sel")
            nc.vector.tensor_copy(h1sel[:], onesw_f[:])

            _stk = ExitStack()
            attnpool = _stk.enter_context(tc.tile_pool(name="attn", bufs=1))
            _qkstk = ExitStack()
            qkpool = _qkstk.enter_context(tc.tile_pool(name="qk", bufs=1))
            vapool = _qkstk.enter_context(tc.tile_pool(name="vaug", bufs=1))

            qT = [qkpool.tile([P, S], bf16, tag=f"qT{i}", name=f"qT{i}")
                  for i in range(4)]
            kT = [qkpool.tile([P, S], bf16, tag=f"kT{i}", name=f"kT{i}")
                  for i in range(4)]
            va = [vapool.tile([P, 8 * 65], bf16, tag=f"va{i}", name=f"va{i}")
                  for i in range(8)]
            attnT = [attnpool.tile([P, S], bf16, tag=f"at{i}", name=f"at{i}")
                     for i in range(4)]

            # [D-half][qc][512 D rows][512 seq]; each RS half contiguous
            po_dram = dram.tile([2, 2, 512, 512], bf16)
            rs_dram = dram.tile([2, 512, 512], bf16)

            # ---- phase 1+2: load xT / weights, project v then q,k ----
            _xwstk = ExitStack()
            xw = _xwstk.enter_context(tc.tile_pool(name="xw", bufs=1))
            with (
                tc.tile_pool(name="proj_ps", bufs=6, space="PSUM") as pps,
                tc.tile_pool(name="rtmp", bufs=4) as rtmp,
            ):
                xt = [xw.tile([P, S], bf16, tag=f"x{i}", name=f"x{i}")
                      for i in range(8)]
                cos_sb = xw.tile([P, S], f32, tag="cos")
                sin_sb = xw.tile([P, S], f32, tag="sin")

                wq_sb = [xw.tile([P, 512], bf16, tag=f"wq{i}", name=f"wqs{i}")
                         for i in range(8)]
                wk_sb = [xw.tile([P, 512], bf16, tag=f"wk{i}", name=f"wks{i}")
                         for i in range(8)]
                wv_sb = [xw.tile([P, 512], bf16, tag=f"wv{i}", name=f"wvs{i}")
                         for i in range(8)]
                for i in range(8):
                    nc.sync.dma_start(xt[i][:], xT[i * P:(i + 1) * P, :])
                for i in range(8):
                    nc.sync.dma_start(wq_sb[i][:], wqT[i * P:(i + 1) * P, :])
                for i in range(8):
                    nc.sync.dma_start(wk_sb[i][:], wkT[i * P:(i + 1) * P, :])
                nc.sync.dma_start(cos_sb[:], cosT[:])
                nc.sync.dma_start(sin_sb[:], sinS[:])
                for i in range(8):
                    nc.sync.dma_start(wv_sb[i][:], wvT[i * P:(i + 1) * P, :])
                nc.sync.dma_start(md_sb[:], mdiag[:])
                nc.sync.dma_start(hz1l_sb[:], hz1l[:])
                nc.sync.dma_start(hz2_sb[:], hz2[:])
                nc.sync.dma_start(hselq_sb[:], hselq[:])
                nc.sync.dma_start(hrstd_sb[:], hrstd[:])
                nc.sync.dma_start(b1_sb[:], b1c[:])
                nc.sync.dma_start(b2_sb[:], b2c[:])
                nc.sync.dma_start(rw_sb[:], rmswc[:])
                for i in range(4):
                    nc.sync.dma_start(wo_sb[i][:],
                                      woT[i * P:(i + 1) * P, :])

                def project_v(st, pool=None, tag="ps"):
                    ps = (pool or pps).tile([P, 512], f32, tag=tag,
                                            name="ps")
                    for kd in range(8):
                        nc.tensor.matmul(
                            ps[:],
                            lhsT=xt[kd][:, st * P:(st + 1) * P],
                            rhs=wv_sb[kd][:],
                            start=(kd == 0), stop=(kd == 7))
                    va3 = va[st][:].rearrange("p (h e) -> p h e", h=8, e=65)
                    nc.vector.tensor_copy(
                        va3[:, :, 0:64],
                        ps[:].rearrange("p (h e) -> p h e", h=8, e=64))
                    nc.vector.tensor_copy(
                        va3[:, :, 64:65],
                        ones_bf[:].rearrange("p (h o) -> p h o", o=1))

                def project_qk(mt):
                    # both seq chunks share each lhsT slice
                    for wsb, dstT in ((wq_sb, qT), (wk_sb, kT)):
                        ps0 = pps.tile([P, 512], f32, tag="ps", name="ps")
                        ps1 = pps.tile([P, 512], f32, tag="ps", name="ps")
                        for kd in range(8):
                            lh = wsb[kd][:, mt * P:(mt + 1) * P]
                            nc.tensor.matmul(ps0[:], lhsT=lh,
                                             rhs=xt[kd][:, 0:512],
                                             start=(kd == 0), stop=(kd == 7))
                            nc.tensor.matmul(ps1[:], lhsT=lh,
                                             rhs=xt[kd][:, 512:1024],
                                             start=(kd == 0), stop=(kd == 7))
                        for nch, ps in ((0, ps0), (1, ps1)):
                            n0 = nch * 512
                            dst = dstT[mt][:, n0:n0 + 512]
                            tmp = rtmp.tile([P, 512], f32, tag="rt",
                                            name="rt")
                            nc.vector.stream_shuffle(tmp[:], ps[:], SWAP16)
                            nc.vector.tensor_tensor(
                                dst, ps[:], cos_sb[:, n0:n0 + 512], Alu.mult)
                            tmp2 = rtmp.tile([P, 512], bf16, tag="rt2",
                                             name="rt2")
                            nc.vector.tensor_tensor(
                                tmp2[:], tmp[:], sin_sb[:, n0:n0 + 512],
                                Alu.mult)
                            nc.gpsimd.tensor_tensor(dst, dst, tmp2[:],
                                                    Alu.add)

                # va[4..7] (k/v seq 512-1023, first used by qc1 kt>=4) are
                # deferred into the qc0 attention window as dense
                # full-array PE work that keeps the HAM un-throttled.
                for mt in range(4):
                    project_qk(mt)
                    project_v(mt)

            # ---- phase 3: differential attention ---------------------
            with (
                tc.tile_pool(name="st_ps", bufs=2, space="PSUM") as stp,
                tc.tile_pool(name="pv_ps", bufs=2, space="PSUM") as pvp,
                tc.tile_pool(name="ms_ps", bufs=1, space="PSUM") as msp,
                tc.tile_pool(name="bc_ps", bufs=1, space="PSUM") as bcp,
                tc.tile_pool(name="epool", bufs=3) as epool,
                tc.tile_pool(name="apool", bufs=3) as apool,
                tc.tile_pool(name="zpool", bufs=2) as zpool,
                tc.tile_pool(name="post", bufs=2) as post,
            ):
                zpt = {}
                ms = {}
                aw = {}

                def attn_pass(qc, pt, h):
                    # one head (2pt+h): branch pair gq=(2h, 2h+1).
                    # scores/exp for kt+1 are emitted BEFORE pv of kt so the
                    # strict-FIFO PE queue always has independent matmuls
                    # ahead of the exp-gated pv pair.
                    q0 = qc * 512
                    kts = _kts(qc)
                    last_kt = kts[-1][0]
                    pvA = pvp.tile([65, 512], f32, tag="pv", name="pvA")
                    pvB = pvp.tile([65, 512], f32, tag="pv", name="pvB")
                    hv = 2 * pt + h
                    es = {}

                    def emit_scores(kt, off):
                        j0 = 0 if off is None else off
                        st = stp.tile([P, 1024], f32, tag="st", name="st")
                        st3 = st[:].rearrange("p (g n) -> p g n", g=2, n=512)
                        for gi, g in enumerate((2 * h, 2 * h + 1)):
                            nc.tensor.matmul(
                                st3[:, gi, j0:],
                                lhsT=kT[pt][g * 32:(g + 1) * 32,
                                            kt * P:(kt + 1) * P],
                                rhs=qT[pt][g * 32:(g + 1) * 32,
                                           q0 + j0:q0 + 512],
                                start=True, stop=True,
                                tile_position=(g * 32, 0))
                        e = epool.tile([P, 1024], bf16, tag="e", name="e")
                        e3 = e[:].rearrange("p (g n) -> p g n", g=2, n=512)
                        nc.scalar.activation(e3[:, :, j0:], st3[:, :, j0:],
                                             Act.Exp, scale=SCALE)
                        if off is not None:
                            nc.vector.tensor_tensor(
                                e3[:, :, j0:j0 + P], e3[:, :, j0:j0 + P],
                                md_sb[:].rearrange("p (g n) -> p g n", g=2),
                                Alu.mult)
                        es[kt] = e3

                    def emit_pv(kt, off):
                        j0 = 0 if off is None else off
                        e3 = es.pop(kt)
                        nc.tensor.matmul(
                            pvA[:, j0:], lhsT=va[kt][:, hv * 65:hv * 65 + 65],
                            rhs=e3[:, 0, j0:],
                            start=(kt == 0), stop=(kt == last_kt))
                        nc.tensor.matmul(
                            pvB[:, j0:], lhsT=va[kt][:, hv * 65:hv * 65 + 65],
                            rhs=e3[:, 1, j0:],
                            start=(kt == 0), stop=(kt == last_kt))

                    emit_scores(*kts[0])
                    for i, (kt, off) in enumerate(kts):
                        if i + 1 < len(kts):
                            emit_scores(*kts[i + 1])
                        emit_pv(kt, off)
                    # evict: A rows (bf16) + Z rows at 32-aligned partitions.
                    # The last pass splits across DVE and the (by then idle)
                    # scalar engine to shorten the tail before wo/RS.
                    last = (qc == 1 and pt == 3)
                    a1, a2 = aw[(qc, pt)]
                    nc.vector.tensor_copy(a1[64 * h:64 * h + 64, :],
                                          pvA[0:64, :])
                    (nc.scalar.copy if last else nc.vector.tensor_copy)(
                        a2[64 * h:64 * h + 64, :], pvB[0:64, :])
                    z = zpt[(qc, pt)]
                    (nc.scalar.copy if last else nc.vector.tensor_copy)(
                        z[32 * h:32 * h + 1, :], pvA[64:65, :])
                    nc.vector.tensor_copy(z[64 + 32 * h:65 + 32 * h, :],
                                          pvB[64:65, :])

                def attn_pt(qc, pt):
                    if (qc, pt) not in aw:
                        aw[(qc, pt)] = (
                            apool.tile([P, 512], bf16, tag="a1", name="a1",
                                       bufs=5),
                            apool.tile([P, 512], bf16, tag="a2", name="a2",
                                       bufs=5))
                        z = zpool.tile([P, 512], f32r, tag="zpt", name="zpt",
                                       bufs=5)
                        nc.vector.memset(z[:].bitcast(f32), 0.0)
                        zpt[(qc, pt)] = z
                    attn_pass(qc, pt, 0)
                    attn_pass(qc, pt, 1)

                def emit_posw(qc, pt):
                    # posw = Z2*A1 - lam*Z1*A2 ; sq = posw^2/8 ; ms rows.
                    # The broadcasts read this pt's zpt directly (K=128
                    # selector) so posw pipelines with the next pt's pass.
                    a1, a2 = aw[(qc, pt)]
                    z = zpt[(qc, pt)]
                    bc2 = bcp.tile([P, 512], f32, tag="bc", name="bc2")
                    nc.tensor.matmul(bc2[:], lhsT=hz2_sb[:],
                                     rhs=z[:], start=True, stop=True)
                    t1 = post.tile([P, 512], f32r, tag="t1")
                    nc.vector.tensor_tensor(t1[:], a1[:], bc2[:], Alu.mult)
                    bc1 = bcp.tile([P, 512], f32, tag="bc", name="bc1")
                    nc.tensor.matmul(bc1[:], lhsT=hz1l_sb[:],
                                     rhs=z[:], start=True, stop=True)
                    t2 = post.tile([P, 512], f32r, tag="t2")
                    nc.vector.tensor_tensor(t2[:], a2[:], bc1[:], Alu.mult)
                    posw = post.tile([P, 512], f32r, tag="posw", bufs=5,
                                     name="posw")
                    nc.vector.tensor_tensor(posw[:], t1[:], t2[:],
                                            Alu.subtract)
                    sq = post.tile([P, 512], f32r, tag="sq", name="sq")
                    nc.vector.scalar_tensor_tensor(
                        sq[:], in0=posw[:], scalar=0.125, in1=posw[:],
                        op0=Alu.mult, op1=Alu.mult)
                    nc.tensor.matmul(ms[qc][:],
                                     lhsT=hselq_sb[:, 8 * pt:8 * pt + 8],
                                     rhs=sq[:], start=(pt == 0),
                                     stop=(pt == 3))
                    return posw

                def emit_rstd(qc):
                    # rstd rows = (1-li)*rsqrt(ms), [8,512]; the reference's
                    # +eps*(z1*z2)^2 term is ~4e-5 relative to ms and the
                    # (1-li) factor folds into the Rsqrt input scale.
                    srt = zpool.tile([8, 512], f32, tag="srt", name="srt")
                    nc.scalar.activation(srt[:], ms[qc][0:8, :], Act.Sqrt,
                                         scale=1.0 / (1.0 - LAMBDA_INIT) ** 2)
                    rstd = zpool.tile([8, 512], f32, tag="rstd", name="rstd")
                    nc.vector.reciprocal_approx_fast(rstd[:], srt[:])
                    rstr = zpool.tile([8, 512], f32r, tag="rstr",
                                      name="rstr")
                    nc.vector.tensor_copy(rstr[:], rstd[:])
                    return rstr[:]

                def emit_apply(qc, pt, posw, rstd):
                    q0 = qc * 512
                    bcr = bcp.tile([P, 512], f32, tag="bc", name="bcr")
                    nc.tensor.matmul(bcr[:],
                                     lhsT=hrstd_sb[:, pt * P:(pt + 1) * P],
                                     rhs=rstd, start=True, stop=True)
                    nc.vector.tensor_tensor(attnT[pt][:, q0:q0 + 512],
                                            posw[:], bcr[:], Alu.mult)

                poswq = {}
                for qc in range(2):
                    ms[qc] = msp.tile([8, 512], f32, tag="ms", name="ms")
                vdefer = {0: (4, 5), 1: (6,), 2: (7,), 3: ()}
                for pt in range(4):
                    attn_pt(0, pt)
                    for st in vdefer[pt]:
                        project_v(st, pool=bcp, tag="bc")
                    if pt > 0:
                        poswq[(0, pt - 1)] = emit_posw(0, pt - 1)
                # ---- phase 4 interleaved: wo by D-half + pair RS -----
                with tc.tile_pool(name="po_sb", bufs=4) as posb:
                    def emit_wo(dh, qc, pool=None):
                        q0 = qc * 512
                        for mo in range(4 * dh, 4 * dh + 4):
                            if pool is None:
                                pst = bcp.tile([P, 512], f32, tag="bc",
                                               name="wops")
                                ps = pst[:]
                            else:
                                pst = pool.tile([P, 1024], f32, tag="st",
                                                name="wops")
                                ps = pst[:, 0:512]
                            for kc in range(4):
                                nc.tensor.matmul(
                                    ps,
                                    lhsT=wo_sb[kc][:, mo * P:(mo + 1) * P],
                                    rhs=attnT[kc][:, q0:q0 + 512],
                                    start=(kc == 0), stop=(kc == 3))
                            po = posb.tile([P, 512], bf16, tag="po")
                            nc.vector.tensor_copy(po[:], ps)
                            nc.sync.dma_start(
                                po_dram[dh, qc,
                                        (mo % 4) * P:(mo % 4 + 1) * P, :],
                                po[:])

                    for pt in range(4):
                        attn_pt(1, pt)
                        if pt == 0:
                            poswq[(0, 3)] = emit_posw(0, 3)
                        else:
                            poswq[(1, pt - 1)] = emit_posw(1, pt - 1)
                        if pt == 1:
                            rstd0 = emit_rstd(0)
                            for p2 in range(4):
                                emit_apply(0, p2, poswq[(0, p2)], rstd0)
                        if pt == 2:
                            emit_wo(0, 0)
                        if pt == 3:
                            emit_wo(1, 0)
                    poswq[(1, 3)] = emit_posw(1, 3)
                    rstd1 = emit_rstd(1)
                    for p2 in range(4):
                        emit_apply(1, p2, poswq[(1, p2)], rstd1)
                    emit_wo(0, 1, pool=stp)
                    nc.gpsimd.collective_compute(
                        "ReduceScatter",
                        mybir.AluOpType.add,
                        replica_groups=[[0, 1], [2, 3], [4, 5], [6, 7]],
                        ins=[po_dram[0].opt()],
                        outs=[rs_dram[0].opt()],
                    )
                    emit_wo(1, 1, pool=stp)
                    nc.gpsimd.collective_compute(
                        "ReduceScatter",
                        mybir.AluOpType.add,
                        replica_groups=[[0, 1], [2, 3], [4, 5], [6, 7]],
                        ins=[po_dram[1].opt()],
                        outs=[rs_dram[1].opt()],
                    )

            _xwstk.close()
            _qkstk.close()
            _stk.close()

            # ---- phase 5: FFN + residual + final RMS on seq shard ----
            with (
                tc.tile_pool(name="aT", bufs=1) as atp,
                tc.tile_pool(name="h1", bufs=1) as h1p,
                tc.tile_pool(name="w1p", bufs=9) as w1p,
                tc.tile_pool(name="w2p", bufs=3) as w2p,
                tc.tile_pool(name="yT", bufs=1) as ytp,
                tc.tile_pool(name="fin", bufs=2) as finp,
                tc.tile_pool(name="sm2", bufs=1) as sm2,
            ):
                aTr = [atp.tile([P, 512], bf16, tag=f"ar{i}", name=f"ar{i}")
                       for i in range(8)]
                for i in range(8):
                    nc.sync.dma_start(
                        aTr[i][:],
                        rs_dram[i // 4, (i % 4) * P:(i % 4 + 1) * P, :])

                h1 = [h1p.tile([P, 512], bf16, tag=f"h1_{i}", name=f"h1_{i}")
                      for i in range(32)]
                with tc.tile_pool(name="h1_ps", bufs=8, space="PSUM") as h1ps:
                    wts = {}
                    pss = {}

                    def h1_first(mf):
                        wt = w1p.tile([P, 1024], bf16, tag="w1t", name="w1t")
                        nc.sync.dma_start(wt[:], w1s[mf, :, :])
                        ps = h1ps.tile([P, 512], f32, tag="h1ps",
                                       name="h1ps")
                        for kd in range(4):
                            nc.tensor.matmul(
                                ps[:], lhsT=wt[:, kd * P:(kd + 1) * P],
                                rhs=aTr[kd][:], start=(kd == 0), stop=False)
                        wts[mf], pss[mf] = wt, ps

                    def h1_second(mf):
                        wt, ps = wts.pop(mf), pss.pop(mf)
                        for kd in range(4, 8):
                            nc.tensor.matmul(
                                ps[:], lhsT=wt[:, kd * P:(kd + 1) * P],
                                rhs=aTr[kd][:], start=False, stop=(kd == 7))
                        nc.scalar.activation(h1[mf][:], ps[:], Act.Relu,
                                             bias=b1_sb[:, mf:mf + 1])

                    for mf in range(8):
                        h1_first(mf)
                    for mf in range(32):
                        h1_second(mf)
                        if mf + 8 < 32:
                            h1_first(mf + 8)

                # h2: 8 persistent PSUM accumulators, stream w2 tiles
                yt = [ytp.tile([P, 512], f32, tag=f"y{i}", name=f"y{i}")
                      for i in range(8)]
                with tc.tile_pool(name="h2_ps", bufs=1, space="PSUM") as h2ps:
                    ps8 = [h2ps.tile([P, 512], f32, tag=f"h2_{mo}",
                                     name=f"h2_{mo}") for mo in range(8)]
                    for kf in range(32):
                        wt2 = w2p.tile([P, 1024], bf16, tag="w2t",
                                       name="w2t")
                        nc.sync.dma_start(wt2[:], w2T[kf * P:(kf + 1) * P, :])
                        for mo in range(8):
                            nc.tensor.matmul(
                                ps8[mo][:], lhsT=wt2[:, mo * P:(mo + 1) * P],
                                rhs=h1[kf][:], start=(kf == 0),
                                stop=(kf == 31))
                    for mo in range(8):
                        nc.vector.scalar_tensor_tensor(
                            yt[mo][:], in0=ps8[mo][:],
                            scalar=b2_sb[:, mo:mo + 1], in1=aTr[mo][:],
                            op0=Alu.add, op1=Alu.add)

                with tc.tile_pool(name="rms_ps", bufs=1,
                                  space="PSUM") as rmsps:
                    ms_ps = rmsps.tile([P, 512], f32, tag="rmsps",
                                       name="rmsps")
                    for mo in range(8):
                        sq = finp.tile([P, 512], f32r, tag="fsq", name="fsq")
                        nc.scalar.activation(sq[:], yt[mo][:], Act.Square)
                        nc.tensor.matmul(ms_ps[0:1, :], lhsT=ones_r[:],
                                         rhs=sq[:], start=(mo == 0),
                                         stop=(mo == 7))
                    srt = sm2.tile([1, 512], f32, tag="fsrt")
                    nc.scalar.activation(srt[:], ms_ps[0:1, :], Act.Sqrt,
                                         scale=1.0 / 1024.0, bias=eps_sb[:])
                    rstd = sm2.tile([1, 512], f32, tag="frstd")
                    nc.vector.reciprocal_approx_fast(rstd[:], srt[:])
                    rstr = sm2.tile([1, 512], f32r, tag="frstr")
                    nc.vector.tensor_copy(rstr[:], rstd[:])
                    bcr = rmsps.tile([P, 512], f32, tag="fbc", name="fbc")
                    nc.tensor.matmul(bcr[:], lhsT=h1sel[:], rhs=rstr[:],
                                     start=True, stop=True)
                    for mo in range(8):
                        ot = finp.tile([P, 512], f32, tag="fot", name="fot")
                        nc.vector.scalar_tensor_tensor(
                            ot[:], in0=yt[mo][:], scalar=rw_sb[:, mo:mo + 1],
                            in1=bcr[:], op0=Alu.mult, op1=Alu.mult)
                        nc.sync.dma_start(outT[mo * P:(mo + 1) * P, :], ot[:])

    nc.compile()
    return nc


def _host_prep(inputs):
    import ml_dtypes
    bfloat16 = ml_dtypes.bfloat16
    x = np.asarray(inputs["x"], dtype=np.float32)
    Wq = np.asarray(inputs["Wq"], dtype=np.float32)
    Wk = np.asarray(inputs["Wk"], dtype=np.float32)
    Wv = np.asarray(inputs["Wv"], dtype=np.float32)
    Wo = np.asarray(inputs["Wo"], dtype=np.float32)
    W1 = np.asarray(inputs["W1"], dtype=np.float32)
    b1 = np.asarray(inputs["b1"], dtype=np.float32)
    W2 = np.asarray(inputs["W2"], dtype=np.float32)
    b2 = np.asarray(inputs["b2"], dtype=np.float32)
    rmsw = np.asarray(inputs["rms_weight"], dtype=np.float32)
    lam = float(np.exp(np.dot(np.asarray(inputs["lambda_q1"], np.float64),
                              np.asarray(inputs["lambda_k1"], np.float64)))
                - np.exp(np.dot(np.asarray(inputs["lambda_q2"], np.float64),
                                np.asarray(inputs["lambda_k2"], np.float64)))
                + LAMBDA_INIT)

    half = HD // 2
    freqs = (1.0 / (10000.0 ** (np.arange(half, dtype=np.float32)
                                / np.float32(half)))).astype(np.float32)
    ang = (np.arange(S, dtype=np.float32)[:, None] * freqs[None, :])
    cos16 = np.cos(ang.astype(np.float32)).T.astype(np.float32)
    sin16 = np.sin(ang.astype(np.float32)).T.astype(np.float32)

    cosT = np.ascontiguousarray(
        np.tile(np.concatenate([cos16, cos16], 0), (4, 1)))
    sinS = np.ascontiguousarray(
        np.tile(np.concatenate([-sin16, sin16], 0), (4, 1))).astype(np.float32)
    perm32 = np.concatenate([np.arange(0, 32, 2), np.arange(1, 32, 2)])

    # multiplicative causal mask for the diag band, tiled for both branches
    md = (np.arange(128)[:, None] <= np.arange(128)[None, :])
    mdiag = np.ascontiguousarray(
        np.tile(md.astype(np.float32), (1, 2)).astype(bfloat16))

    # selector matmuls: bc[p,s] = sum_k lhsT[k,p] * zq[k,s]
    # zq rows 0..7 = Z1 (key 2pt+h), 8..15 = Z2
    hz1l = np.zeros((128, 128), np.float32)
    hz2 = np.zeros((128, 128), np.float32)
    hselq = np.zeros((128, 32), np.float32)
    hrstd = np.zeros((8, 512), np.float32)
    for h in range(2):
        # bc[p, s] = zpt[32h(p), s]; zpt row 32h = Z1(h), 64+32h = Z2(h)
        pcols = slice(64 * h, 64 * h + 64)
        hz1l[32 * h, pcols] = lam
        hz2[64 + 32 * h, pcols] = 1.0
    for pt in range(4):
        for h in range(2):
            rows = slice(64 * h, 64 * h + 64)
            cols = slice(pt * 128 + 64 * h, pt * 128 + 64 * h + 64)
            hselq[rows, 8 * pt + 2 * pt + h] = 0.125
            hrstd[2 * pt + h, cols] = 1.0

    b1c = np.ascontiguousarray(b1.reshape(32, 128).T)
    b2c = np.ascontiguousarray(b2.reshape(8, 128).T)
    rmswc = np.ascontiguousarray(rmsw.reshape(8, 128).T)
    # w1s[mf][p, kd*128+j] = W1.T[kd*128+p, mf*128+j]
    w1s = np.ascontiguousarray(
        W1.T.reshape(8, 128, 32, 128).transpose(2, 1, 0, 3)
        .reshape(32, 128, 1024).astype(bfloat16))
    w2T = np.ascontiguousarray(W2.T.astype(bfloat16))

    in_maps = []
    for c in range(NCORES):
        b, g = c // 2, c % 2
        chans = np.arange(g * 512, (g + 1) * 512)
        permed = np.concatenate(
            [c0 * 32 + perm32 for c0 in range(g * 16, (g + 1) * 16)])
        in_maps.append({
            "xT": np.ascontiguousarray(x[b].T.astype(bfloat16)),
            "wqT": np.ascontiguousarray(Wq[permed, :].T.astype(bfloat16)),
            "wkT": np.ascontiguousarray(Wk[permed, :].T.astype(bfloat16)),
            "wvT": np.ascontiguousarray(Wv[chans, :].T.astype(bfloat16)),
            "woT": np.ascontiguousarray(Wo[:, chans].T.astype(bfloat16)),
            "w1s": w1s, "w2T": w2T,
            "b1c": b1c, "b2c": b2c, "rmswc": rmswc,
            "cosT": cosT, "sinS": sinS,
            "mdiag": mdiag,
            "hz1l": hz1l, "hz2": hz2, "hselq": hselq, "hrstd": hrstd,
        })
    return in_maps


def kernel(**inputs):
    global LAST_RESULT
    from concourse.bass_utils import run_bass_kernel_spmd

    if "nc" not in _PROGRAM:
        _PROGRAM["nc"] = _build_program()
    nc = _PROGRAM["nc"]

    in_maps = _host_prep(inputs)
    trace = bool(int(os.environ.get("KERNEL_TRACE", "0")))
    res = run_bass_kernel_spmd(nc, in_maps, list(range(NCORES)), trace=trace)
    LAST_RESULT = res

    out = np.empty((B, S, D), np.float32)
    for c in range(NCORES):
        b, g = c // 2, c % 2
        out[b, g * 512:(g + 1) * 512, :] = res.results[c]["outT"].T
    return out

